# revision 1
# baseline (speedup 1.0000x reference)
"""Trainium2 Bass kernel for nn_DownModel (GNN message passing + kNN graph + GCN).

Math (from the reference):
  f1   = elu(c00*global*relu(pre_token*feat) + c01*pre_token_w*feat)      [N,H]
  agg  = scatter-add over E edges of adj_vals * f1[adj_cols]              [N,H]
  y    = f1 @ gcn_w                                                       [N,C]
  emb  = normalize_rows(balance_w * [f1 | agg])                           [N,2H]
  sim  = emb @ emb.T ; vals,idx = top_k(sim, 16)                          [N,N]
  out  = relu(0.5*(A @ y) + 0.5*sum_j relu(vals_j)*y[idx_j] + b)          [N,C]

Sharding: row-block N across 8 cores.  Each core computes f1/y for its rows,
all-gathers the [N, H+C] table, does the edge segment-sum for its rows via
one-hot-selector matmuls (edges bucketed host-side by destination row chunk),
all-gathers the transposed embedding, computes its [N/8, N] block of sim,
top-k's each row on the vector engine, and combines.  new_adj is never
materialized: new_adj @ y = 0.5*SpMM_edges(y) + 0.5*topk-weighted-gather(y).
"""

import numpy as np

import concourse.bass as bass
import concourse.mybir as mybir
import concourse.tile as tile
from concourse.bass_utils import run_bass_kernel_spmd

F32 = mybir.dt.float32
I32 = mybir.dt.int32
U32 = mybir.dt.uint32

N = 8192
H = 256
C_OUT = 40
K_TOP = 16
NCORES = 8
P = 128
ROWS_PER_CORE = N // NCORES          # 1024
CHUNKS = ROWS_PER_CORE // P          # 8
KT = (2 * H) // P                    # 4 k-tiles of the 2H embedding dim
HKT = H // P                         # 2 k-tiles of the H dim
TBLW = H + C_OUT                     # 296 table row width
SIMW = 512                           # sim column chunk width (one PSUM bank)
NCC = N // SIMW                      # 16 column chunks
GROUP = 4                            # row-tiles sharing one rhs stream pass


def _split_waits(nc, maxw=1):
    """This container's walrus only accepts one sync-wait command per
    instruction; hoist excess waits onto preceding same-engine NOPs."""
    n_new = 0
    for bb in nc.main_func.blocks:
        new_insts = []
        for ins in bb.instructions:
            si = ins.sync_info
            if si is not None and si.on_wait and len(si.on_wait) > maxw:
                waits = list(si.on_wait)
                excess, keep = waits[:-maxw], waits[-maxw:]
                for i in range(0, len(excess), maxw):
                    nop = mybir.InstNoOp(
                        name=f"waitnop-{ins.name}-{i}",
                        engine=ins.engine,
                        ins=[],
                        outs=[],
                        sync_info=mybir.SyncInfo(
                            on_wait=excess[i:i + maxw], on_update=[]
                        ),
                    )
                    new_insts.append(nop)
                    n_new += 1
                si.on_wait = keep
            new_insts.append(ins)
        bb.instructions[:] = new_insts
    return n_new


def build(T, sim_dtype=F32, n=N, debug=False, repeat=1, phase_lim=6):
    """Build the SPMD program (identical on all cores; data differs).

    T: number of 128-edge tiles per destination row chunk (padded, global max).
    Execution cost here is dominated by per-instruction dispatch, so ops are
    batched across row-tiles/edge-tiles wherever the ISA allows.
    """
    rows_per_core = n // NCORES
    chunks = rows_per_core // P
    ncc = n // SIMW
    ccg_n = 4                    # sim column chunks per PSUM macro-tile
    nccg = ncc // ccg_n
    ngroups = (chunks + GROUP - 1) // GROUP

    nc = bass.Bass(num_devices=NCORES)
    if debug:
        dbg_table = nc.dram_tensor("dbg_table", [n, TBLW], F32,
                                   kind="ExternalOutput")
        dbg_agg = nc.dram_tensor("dbg_agg", [P, chunks * TBLW], F32,
                                 kind="ExternalOutput")
        dbg_embT = nc.dram_tensor("dbg_embT", [NCORES * 2 * H, rows_per_core],
                                  F32, kind="ExternalOutput")
        dbg_m16 = nc.dram_tensor("dbg_m16", [P, chunks * K_TOP], F32,
                                 kind="ExternalOutput")
        dbg_i16 = nc.dram_tensor("dbg_i16", [P, chunks * K_TOP], U32,
                                 kind="ExternalOutput")

    # ---- per-core inputs ----
    feat_d = nc.dram_tensor("feat", [rows_per_core, H], F32, kind="ExternalInput")
    erow_d = nc.dram_tensor("erow", [chunks, P, T], F32, kind="ExternalInput")
    ecol_d = nc.dram_tensor("ecol", [chunks, P, T], I32, kind="ExternalInput")
    eval_d = nc.dram_tensor("eval", [chunks, P, T], F32, kind="ExternalInput")
    # replicated small params (host pre-broadcast across partitions)
    cA_d = nc.dram_tensor("cA", [P, H], F32, kind="ExternalInput")
    cB_d = nc.dram_tensor("cB", [P, H], F32, kind="ExternalInput")
    cC_d = nc.dram_tensor("cC", [P, H], F32, kind="ExternalInput")
    bal_d = nc.dram_tensor("bal", [P, 2 * H], F32, kind="ExternalInput")
    bias_d = nc.dram_tensor("bias", [P, C_OUT], F32, kind="ExternalInput")
    gcnw_d = nc.dram_tensor("gcnw", [H, C_OUT], F32, kind="ExternalInput")
    iota_d = nc.dram_tensor("iota", [P, P], F32, kind="ExternalInput")
    ident_d = nc.dram_tensor("ident", [P, P], F32, kind="ExternalInput")

    out_d = nc.dram_tensor("out", [rows_per_core, C_OUT], F32, kind="ExternalOutput")

    with tile.TileContext(nc) as tc:
        with tc.tile_pool(name="consts", bufs=1) as cp, \
             tc.tile_pool(name="persist", bufs=1) as pp, \
             tc.tile_pool(name="dram", bufs=1, space="DRAM") as dp:

            # ---- constants into SBUF ----
            cA = cp.tile([P, H], F32)
            cB = cp.tile([P, H], F32)
            cC = cp.tile([P, H], F32)
            bal = cp.tile([P, 2 * H], F32)
            bias = cp.tile([P, C_OUT], F32)
            gcnw = cp.tile([P, HKT * C_OUT], F32)
            iota = cp.tile([P, P], F32)
            ident = cp.tile([P, P], F32)
            nc.sync.dma_start(out=cA[:], in_=cA_d[:])
            nc.sync.dma_start(out=cB[:], in_=cB_d[:])
            nc.sync.dma_start(out=cC[:], in_=cC_d[:])
            nc.sync.dma_start(out=bal[:], in_=bal_d[:])
            nc.sync.dma_start(out=bias[:], in_=bias_d[:])
            for k in range(HKT):
                nc.sync.dma_start(out=gcnw[:, k * C_OUT:(k + 1) * C_OUT],
                                  in_=gcnw_d[k * P:(k + 1) * P, :])
            nc.sync.dma_start(out=iota[:], in_=iota_d[:])
            nc.sync.dma_start(out=ident[:], in_=ident_d[:])

            # ---- persistent SBUF tensors ----
            agg_all = pp.tile([P, chunks * TBLW], F32)      # [agg | A@y] per chunk
            embTloc = pp.tile([P, KT * rows_per_core], F32)  # local emb, transposed

            for rep in range(repeat):
                # ---- DRAM scratch (per rep: Shared tensors are
                # single-writer) ----
                table_loc = dp.tile([rows_per_core, TBLW], F32,
                                    name=f"table_loc_{rep}")
                table_g = dp.tile([n, TBLW], F32, addr_space="Shared",
                                  name=f"table_g_{rep}")
                embT_loc_d = dp.tile([2 * H, rows_per_core], F32,
                                     name=f"embT_loc_d_{rep}")
                embT_g = dp.tile([NCORES * 2 * H, rows_per_core], F32,
                                 addr_space="Shared", name=f"embT_g_{rep}")

                # f1 lives only through P3; manual pool scope frees its
                # SBUF before the P4 sim buffers open.
                _f1ctx = tc.tile_pool(name=f"f1p_{rep}", bufs=1)
                f1p = _f1ctx.__enter__()
                f1_all = f1p.tile([P, chunks * H], F32, name=f"f1_all_{rep}")
                # ===== P1: f1 + y for the local row block (batched) =====
                with tc.tile_pool(name=f"p1_{rep}", bufs=1) as p1, \
                     tc.tile_pool(name=f"p1s_{rep}", bufs=3) as p1s, \
                     tc.tile_pool(name=f"p1ps_{rep}", bufs=2, space="PSUM") as p1ps:
                    W1 = chunks * H
                    ft = p1.tile([P, W1], F32)
                    nc.sync.dma_start(
                        out=ft[:].rearrange("p (r w) -> p r w", r=chunks),
                        in_=feat_d[:].rearrange("(r p) w -> p r w", p=P))
                    mB = lambda c: c[:, None, :].to_broadcast([P, chunks, H])
                    v3 = lambda t: t[:].rearrange("p (r w) -> p r w", r=chunks)
                    m = p1.tile([P, W1], F32)
                    nc.vector.tensor_tensor(out=v3(m), in0=v3(ft), in1=mB(cB),
                                            op=mybir.AluOpType.mult)
                    nc.vector.tensor_scalar(out=m[:], in0=m[:], scalar1=0.0,
                                            scalar2=None, op0=mybir.AluOpType.max)
                    nc.vector.tensor_tensor(out=v3(m), in0=v3(m), in1=mB(cA),
                                            op=mybir.AluOpType.mult)
                    v = p1.tile([P, W1], F32)
                    nc.vector.tensor_tensor(out=v3(v), in0=v3(ft), in1=mB(cC),
                                            op=mybir.AluOpType.mult)
                    nc.vector.tensor_tensor(out=m[:], in0=m[:], in1=v[:],
                                            op=mybir.AluOpType.add)
                    # elu(z) = (relu(z) - 1) + exp(min(z, 0))
                    nc.vector.tensor_scalar(out=v[:], in0=m[:], scalar1=0.0,
                                            scalar2=-1.0, op0=mybir.AluOpType.max,
                                            op1=mybir.AluOpType.add)
                    nc.vector.tensor_scalar(out=m[:], in0=m[:], scalar1=0.0,
                                            scalar2=None, op0=mybir.AluOpType.min)
                    nc.scalar.activation(out=m[:], in_=m[:],
                                         func=mybir.ActivationFunctionType.Exp)
                    nc.vector.tensor_tensor(out=f1_all[:], in0=v[:], in1=m[:],
                                            op=mybir.AluOpType.add)
                    nc.sync.dma_start(
                        out=table_loc[:, 0:H].rearrange("(r p) w -> p r w", p=P),
                        in_=v3(f1_all))
                    # y = f1 @ gcn_w  (transpose f1 k-blocks, then matmul;
                    # all row-tiles accumulate into one PSUM bank)
                    psy = p1ps.tile([P, chunks * C_OUT], F32, space="PSUM",
                                    name="psy_all")
                    for rt in range(chunks):
                        f1T = p1s.tile([P, HKT * P], F32)
                        psT = p1ps.tile([P, HKT * P], F32, space="PSUM")
                        for k in range(HKT):
                            nc.tensor.transpose(
                                out=psT[:, k * P:(k + 1) * P],
                                in_=f1_all[:, rt * H + k * P:rt * H + (k + 1) * P],
                                identity=ident[:])
                        nc.scalar.copy(out=f1T[:], in_=psT[:])
                        for k in range(HKT):
                            nc.tensor.matmul(out=psy[:, rt * C_OUT:(rt + 1) * C_OUT],
                                             lhsT=f1T[:, k * P:(k + 1) * P],
                                             rhs=gcnw[:, k * C_OUT:(k + 1) * C_OUT],
                                             start=(k == 0), stop=(k == HKT - 1))
                    yt = p1s.tile([P, chunks * C_OUT], F32, name="yt_all")
                    nc.scalar.copy(out=yt[:], in_=psy[:])
                    nc.sync.dma_start(
                        out=table_loc[:, H:TBLW].rearrange("(r p) w -> p r w", p=P),
                        in_=yt[:].rearrange("p (r w) -> p r w", r=chunks))

                if phase_lim >= 2:
                    # ===== P1b: all-gather the [n, H+C] table =====
                    nc.gpsimd.collective_compute(
                        "AllGather", mybir.AluOpType.bypass,
                        replica_groups=[list(range(NCORES))],
                        ins=[table_loc.opt()], outs=[table_g.opt()],
                    )
                    if debug and rep == 0:
                        nc.sync.dma_start(out=dbg_table[:], in_=table_g[:])

                if phase_lim >= 3:
                    # ===== P2: edge segment-sum (agg | A@y) =====
                    with tc.tile_pool(name=f"p2e_{rep}", bufs=4) as p2e, \
                         tc.tile_pool(name=f"p2g_{rep}", bufs=2) as p2g, \
                         tc.tile_pool(name=f"p2s_{rep}", bufs=2) as p2s, \
                         tc.tile_pool(name=f"p2ps_{rep}", bufs=2, space="PSUM") as p2ps:
                        for ci in range(chunks):
                            er = p2e.tile([P, T], F32)
                            ec = p2e.tile([P, T], I32)
                            ev = p2e.tile([P, T], F32)
                            nc.sync.dma_start(out=er[:], in_=erow_d[ci])
                            nc.sync.dma_start(out=ec[:], in_=ecol_d[ci])
                            nc.sync.dma_start(out=ev[:], in_=eval_d[ci])
                            # one-hot selector blocks for all T tiles: 2 ops
                            S_all = p2s.tile([P, T * P], F32)
                            S3 = S_all[:].rearrange("p (t r) -> p t r", t=T)
                            nc.vector.tensor_tensor(
                                out=S3,
                                in0=er[:, :, None].to_broadcast([P, T, P]),
                                in1=iota[:, None, :].to_broadcast([P, T, P]),
                                op=mybir.AluOpType.is_equal)
                            nc.vector.tensor_tensor(
                                out=S3, in0=S3,
                                in1=ev[:, :, None].to_broadcast([P, T, P]),
                                op=mybir.AluOpType.mult)
                            psa = p2ps.tile([P, TBLW], F32, space="PSUM")
                            gb = p2g.tile([P, T * TBLW], F32, tag="gtile",
                                          name=f"g_{ci}")
                            for t in range(T):
                                nc.gpsimd.indirect_dma_start(
                                    out=gb[:, t * TBLW:(t + 1) * TBLW],
                                    out_offset=None,
                                    in_=table_g[:, :],
                                    in_offset=bass.IndirectOffsetOnAxis(
                                        ap=ec[:, t:t + 1], axis=0),
                                )
                            for t in range(T):
                                nc.tensor.matmul(out=psa[:],
                                                 lhsT=S_all[:, t * P:(t + 1) * P],
                                                 rhs=gb[:, t * TBLW:(t + 1) * TBLW],
                                                 start=(t == 0), stop=(t == T - 1))
                            nc.scalar.copy(out=agg_all[:, ci * TBLW:(ci + 1) * TBLW],
                                           in_=psa[:])
                    if debug and rep == 0:
                        nc.sync.dma_start(out=dbg_agg[:], in_=agg_all[:])

                if phase_lim >= 4:
                    # ===== P3: embedding build + transpose (batched) =====
                    with tc.tile_pool(name=f"p3_{rep}", bufs=1) as p3, \
                         tc.tile_pool(name=f"p3ps_{rep}", bufs=2, space="PSUM") as p3ps:
                        W3 = chunks * 2 * H
                        zc = p3.tile([P, W3], F32)
                        zc3 = zc[:].rearrange("p (r w) -> p r w", r=chunks)
                        nc.vector.tensor_tensor(
                            out=zc3[:, :, 0:H],
                            in0=f1_all[:].rearrange("p (r w) -> p r w", r=chunks),
                            in1=bal[:, None, 0:H].to_broadcast([P, chunks, H]),
                            op=mybir.AluOpType.mult)
                        nc.vector.tensor_tensor(
                            out=zc3[:, :, H:2 * H],
                            in0=agg_all[:].rearrange(
                                "p (r w) -> p r w", r=chunks)[:, :, 0:H],
                            in1=bal[:, None, H:2 * H].to_broadcast([P, chunks, H]),
                            op=mybir.AluOpType.mult)
                        sq = p3.tile([P, W3], F32)
                        n2 = p3.tile([P, chunks], F32)
                        for rt in range(chunks):
                            # Square + row-sum fused on the (idle) scalar
                            # engine; keeps the 4MB reduce off the DVE.
                            nc.scalar.activation(
                                out=sq[:, rt * 2 * H:(rt + 1) * 2 * H],
                                in_=zc[:, rt * 2 * H:(rt + 1) * 2 * H],
                                func=mybir.ActivationFunctionType.Square,
                                accum_out=n2[:, rt:rt + 1])
                        nc.scalar.sqrt(out=n2[:], in_=n2[:])
                        nc.vector.tensor_scalar(out=n2[:], in0=n2[:], scalar1=1e-8,
                                                scalar2=None, op0=mybir.AluOpType.add)
                        inv = p3.tile([P, chunks], F32)
                        nc.vector.reciprocal(out=inv[:], in_=n2[:])
                        nc.vector.tensor_tensor(
                            out=zc3, in0=zc3,
                            in1=inv[:, :, None].to_broadcast([P, chunks, 2 * H]),
                            op=mybir.AluOpType.mult)
                        for rt in range(chunks):
                            psT = p3ps.tile([P, KT * P], F32, space="PSUM")
                            for k in range(KT):
                                nc.tensor.transpose(
                                    out=psT[:, k * P:(k + 1) * P],
                                    in_=zc[:, rt * 2 * H + k * P:
                                           rt * 2 * H + (k + 1) * P],
                                    identity=ident[:])
                            dst3 = embTloc[:].rearrange(
                                "p (k r) -> p k r", k=KT)[:, :, rt * P:(rt + 1) * P]
                            nc.scalar.copy(out=dst3, in_=psT[:].rearrange(
                                "p (k r) -> p k r", k=KT))
                            nc.sync.dma_start(
                                out=embT_loc_d[:, rt * P:(rt + 1) * P].rearrange(
                                    "(k p) r -> p k r", p=P),
                                in_=dst3)

                    # ===== P3b: all-gather transposed embedding =====
                    nc.gpsimd.collective_compute(
                        "AllGather", mybir.AluOpType.bypass,
                        replica_groups=[list(range(NCORES))],
                        ins=[embT_loc_d.opt()], outs=[embT_g.opt()],
                    )
                    if debug and rep == 0:
                        nc.sync.dma_start(out=dbg_embT[:], in_=embT_g[:])

                _f1ctx.__exit__(None, None, None)

                if phase_lim >= 5:
                    # ===== P4+P5: sim row-block, top-k, combine =====
                    cpb = max(1, rows_per_core // SIMW)
                    bw = cpb * SIMW
                    nblk = ccg_n // cpb
                    with tc.tile_pool(name=f"p4rhs_{rep}", bufs=nblk) as p4rhs, \
                         tc.tile_pool(name=f"p4sim_{rep}", bufs=GROUP) as p4sim, \
                         tc.tile_pool(name=f"p4s_{rep}", bufs=2) as p4s, \
                         tc.tile_pool(name=f"p4ps_{rep}", bufs=2, space="PSUM") as p4ps:
                        ot_all = p4s.tile([P, chunks * C_OUT], F32,
                                          bufs=1, name=f"ot_all_{rep}")
                        for g in range(ngroups):
                            rts = [g * GROUP + j for j in range(GROUP)
                                   if g * GROUP + j < chunks]
                            sims = {}
                            for rt in rts:
                                sims[rt] = p4sim.tile([P, n], sim_dtype,
                                                      tag="simbuf",
                                                      name=f"sim_rt{rt}")
                            for cg in range(nccg):
                                halves = []
                                for hf in range(nblk):
                                    blk = cg * nblk + hf
                                    rh = p4rhs.tile([P, KT * bw],
                                                    sim_dtype, tag="rhs",
                                                    name=f"rhs{cg}_{hf}")
                                    nc.sync.dma_start(
                                        out=rh[:].rearrange(
                                            "p (k w) -> p k w", k=KT),
                                        in_=embT_g[blk * 2 * H:(blk + 1) * 2 * H,
                                                   0:bw].rearrange(
                                            "(k p) w -> p k w", p=P))
                                    halves.append(rh)
                                rhss = []
                                for ccq in range(ccg_n):
                                    rhss.append((halves[ccq // cpb], ccq % cpb))
                                for rt in rts:
                                    pss = p4ps.tile([P, ccg_n * SIMW], F32,
                                                    space="PSUM", tag="pss",
                                                    name=f"pss{rt}")
                                    for ccq in range(ccg_n):
                                        for k in range(KT):
                                            nc.tensor.matmul(
                                                out=pss[:, ccq * SIMW:
                                                        (ccq + 1) * SIMW],
                                                lhsT=embTloc[
                                                    :, k * rows_per_core + rt * P:
                                                    k * rows_per_core + (rt + 1) * P],
                                                rhs=rhss[ccq][0][
                                                :, k * bw
                                                + rhss[ccq][1] * SIMW:
                                                k * bw
                                                + (rhss[ccq][1] + 1) * SIMW],
                                                start=(k == 0), stop=(k == KT - 1))
                                    nc.scalar.copy(
                                        out=sims[rt][:, cg * ccg_n * SIMW:
                                                     (cg + 1) * ccg_n * SIMW],
                                        in_=pss[:])
                            for rt in rts:
                                if phase_lim < 6:
                                    mx = p4s.tile([P, 8], F32, name=f"mx{rt}")
                                    nc.vector.max(out=mx[:], in_=sims[rt][:])
                                    continue
                                sim = sims[rt]
                                m16 = p4s.tile([P, K_TOP], F32, tag="m16",
                                               bufs=chunks, name=f"m16_{rt}")
                                i16 = p4s.tile([P, K_TOP], U32, tag="i16",
                                               bufs=chunks, name=f"i16_{rt}")
                                nc.vector.max(out=m16[:, 0:8], in_=sim[:])
                                nc.vector.max_index(out=i16[:, 0:8],
                                                    in_max=m16[:, 0:8],
                                                    in_values=sim[:])
                                nc.vector.match_replace(out=sim[:],
                                                        in_to_replace=m16[:, 0:8],
                                                        in_values=sim[:],
                                                        imm_value=-1e30)
                                nc.vector.max(out=m16[:, 8:16], in_=sim[:])
                                nc.vector.max_index(out=i16[:, 8:16],
                                                    in_max=m16[:, 8:16],
                                                    in_values=sim[:])
                                if debug and rep == 0:
                                    nc.sync.dma_start(
                                        out=dbg_m16[:, rt * K_TOP:(rt + 1) * K_TOP],
                                        in_=m16[:])
                                    nc.sync.dma_start(
                                        out=dbg_i16[:, rt * K_TOP:(rt + 1) * K_TOP],
                                        in_=i16[:])
                                # P5: out2 = sum_j relu(v_j) * y[idx_j]
                                v16 = p4s.tile([P, K_TOP], F32, tag="v16",
                                               bufs=chunks, name=f"v16_{rt}")
                                nc.vector.tensor_scalar(out=v16[:], in0=m16[:],
                                                        scalar1=0.0, scalar2=None,
                                                        op0=mybir.AluOpType.max)
                                y16 = p4s.tile([P, K_TOP * C_OUT], F32)
                                for j in range(K_TOP):
                                    nc.gpsimd.indirect_dma_start(
                                        out=y16[:, j * C_OUT:(j + 1) * C_OUT],
                                        out_offset=None,
                                        in_=table_g[:, :],
                                        in_offset=bass.IndirectOffsetOnAxis(
                                            ap=i16[:, j:j + 1], axis=0),
                                        element_offset=H,
                                    )
                                y16s = p4s.tile([P, K_TOP * C_OUT], F32)
                                nc.scalar.copy(out=y16s[:], in_=y16[:])
                                nc.vector.tensor_tensor(
                                    out=y16s[:].rearrange("p (a b) -> p a b",
                                                          a=K_TOP),
                                    in0=y16s[:].rearrange("p (a b) -> p a b",
                                                          a=K_TOP),
                                    in1=v16[:, :, None].to_broadcast(
                                        [P, K_TOP, C_OUT]),
                                    op=mybir.AluOpType.mult)
                                nc.vector.tensor_reduce(
                                    out=ot_all[:, rt * C_OUT:(rt + 1) * C_OUT],
                                    in_=y16s[:].rearrange("p (a b) -> p b a",
                                                          a=K_TOP),
                                    axis=mybir.AxisListType.X,
                                    op=mybir.AluOpType.add)

                        if phase_lim >= 6:
                            # batched: out = relu(0.5*(out1 + out2) + bias)
                            o3 = ot_all[:].rearrange("p (r w) -> p r w", r=chunks)
                            nc.vector.tensor_tensor(
                                out=o3, in0=o3,
                                in1=agg_all[:].rearrange(
                                    "p (r w) -> p r w", r=chunks)[:, :, H:TBLW],
                                op=mybir.AluOpType.add)
                            nc.vector.tensor_scalar(out=ot_all[:], in0=ot_all[:],
                                                    scalar1=0.5, scalar2=None,
                                                    op0=mybir.AluOpType.mult)
                            nc.vector.tensor_tensor(
                                out=o3, in0=o3,
                                in1=bias[:, None, :].to_broadcast(
                                    [P, chunks, C_OUT]),
                                op=mybir.AluOpType.add)
                            nc.vector.tensor_scalar(out=ot_all[:], in0=ot_all[:],
                                                    scalar1=0.0, scalar2=None,
                                                    op0=mybir.AluOpType.max)
                            nc.sync.dma_start(
                                out=out_d[:].rearrange("(r p) w -> p r w", p=P),
                                in_=o3)

            if phase_lim < 6:
                with tc.tile_pool(name="dummyout", bufs=1) as dop:
                    zz = dop.tile([P, C_OUT], F32)
                    nc.vector.memset(zz[:], 0.0)
                    for rt in range(chunks):
                        nc.sync.dma_start(out=out_d[rt * P:(rt + 1) * P, :],
                                          in_=zz[:])

    return nc


def prep_inputs(features, adj_rows, adj_cols, adj_vals, tokens, wp_weight,
                global_token, pre_token_w, combine_w, balance_w, gcn_w, gcn_b,
                n=N):
    """Host-side sharding: row-block features, bucket edges by destination
    row chunk, pre-broadcast the small parameters."""
    rows_per_core = n // NCORES
    chunks = rows_per_core // P

    features = np.ascontiguousarray(np.asarray(features, dtype=np.float32))
    r = np.asarray(adj_rows).astype(np.int64)
    c = np.asarray(adj_cols).astype(np.int64)
    v = np.asarray(adj_vals, dtype=np.float32)

    pre_token = (np.asarray(wp_weight, np.float32) @
                 np.asarray(tokens, np.float32)).reshape(-1)       # [H]
    cw = np.asarray(combine_w, np.float32).reshape(-1)
    cA = (cw[0] * np.asarray(global_token, np.float32)).reshape(-1)
    cB = pre_token
    cC = (cw[1] * np.asarray(pre_token_w, np.float32)).reshape(-1)
    bal = np.asarray(balance_w, np.float32).reshape(-1)
    bias = np.asarray(gcn_b, np.float32).reshape(-1)

    bcast = lambda x: np.ascontiguousarray(np.tile(x[None, :], (P, 1)))
    cA_b, cB_b, cC_b = bcast(cA), bcast(cB), bcast(cC)
    bal_b, bias_b = bcast(bal), bcast(bias)
    gcnw = np.ascontiguousarray(np.asarray(gcn_w, np.float32))
    iota = np.tile(np.arange(P, dtype=np.float32)[None, :], (P, 1))
    ident = np.eye(P, dtype=np.float32)

    # bucket edges by (core, chunk); sort key = global chunk id
    gchunk = r // P                       # 0 .. n/P-1
    order = np.argsort(gchunk, kind="stable")
    rs, cs, vs = r[order], c[order], v[order]
    gs = gchunk[order]
    counts = np.bincount(gs, minlength=n // P)
    T = max(1, int(np.ceil(counts.max() / P)))

    erow = np.full((NCORES, chunks, T * P), -1.0, dtype=np.float32)
    ecol = np.zeros((NCORES, chunks, T * P), dtype=np.int32)
    evalv = np.zeros((NCORES, chunks, T * P), dtype=np.float32)
    starts = np.concatenate([[0], np.cumsum(counts)])
    for g in range(n // P):
        core, ci = g // chunks, g % chunks
        s, e = starts[g], starts[g + 1]
        cnt = e - s
        erow[core, ci, :cnt] = (rs[s:e] % P).astype(np.float32)
        ecol[core, ci, :cnt] = cs[s:e].astype(np.int32)
        evalv[core, ci, :cnt] = vs[s:e]
    # [chunks, T*P] -> [chunks, P, T] with edge t*P+p at [p, t]
    def shuffle(a):
        return np.ascontiguousarray(
            a.reshape(NCORES, chunks, T, P).transpose(0, 1, 3, 2))
    erow, ecol, evalv = shuffle(erow), shuffle(ecol), shuffle(evalv)

    in_maps = []
    for core in range(NCORES):
        in_maps.append({
            "feat": features[core * rows_per_core:(core + 1) * rows_per_core],
            "erow": erow[core], "ecol": ecol[core], "eval": evalv[core],
            "cA": cA_b, "cB": cB_b, "cC": cC_b, "bal": bal_b, "bias": bias_b,
            "gcnw": gcnw, "iota": iota, "ident": ident,
        })
    return in_maps, T


_BUILD_CACHE = {}


def kernel(features, adj_rows, adj_cols, adj_vals, down_k,
           tokens, wp_weight, global_token, pre_token_w, combine_w,
           balance_w, gcn_w, gcn_b):
    k = int(np.asarray(down_k))
    assert k == K_TOP, f"kernel hardcodes top-k={K_TOP}, got {k}"
    in_maps, T = prep_inputs(features, adj_rows, adj_cols, adj_vals, tokens,
                             wp_weight, global_token, pre_token_w, combine_w,
                             balance_w, gcn_w, gcn_b)
    if T not in _BUILD_CACHE:
        nc_new = build(T)
        _split_waits(nc_new)   # hardware-only fixup; breaks CoreSim if applied
        _BUILD_CACHE[T] = nc_new
    nc = _BUILD_CACHE[T]
    res = run_bass_kernel_spmd(nc, in_maps, list(range(NCORES)))
    out = np.concatenate([res.results[i]["out"] for i in range(NCORES)], axis=0)
    return out.astype(np.float32)



# revision 7
# speedup vs baseline: 48.0138x; 48.0138x over previous
"""Trainium2 Bass kernel for nn_DownModel (GNN message passing + kNN graph + GCN).

Math (from the reference):
  f1   = elu(c00*global*relu(pre_token*feat) + c01*pre_token_w*feat)      [N,H]
  agg  = scatter-add over E edges of adj_vals * f1[adj_cols]              [N,H]
  emb  = normalize_rows(balance_w * [f1 | agg])                           [N,2H]
  sim  = emb @ emb.T ; vals,idx = top_k(sim, 16)                          [N,N]
  g2   = sum_j relu(vals_j) * f1[idx_j]                                   [N,H]
  out  = relu(0.5*(agg + g2) @ gcn_w + b)                                 [N,C]
  (uses A@y = (A@f1)@W and re@y = (re@f1)@W to defer the GCN matmul)

This environment is instruction-dispatch-bound (~25-90us per instruction
regardless of size), so the design minimizes instruction count:
  - segment-sum = one 64-index gather + scale + strided tensor_reduce per
    128-row chunk (edges padded per *row* to T slots host-side)
  - top-k gather = one 16-index gather per row-tile
  - sim row-block matmul in f32 (low precision provably flips top-16
    membership and blows the tolerance)
"""

import numpy as np

import concourse.bass as bass
import concourse.mybir as mybir
import concourse.tile as tile
from concourse.bass_utils import run_bass_kernel_spmd

F32 = mybir.dt.float32
I32 = mybir.dt.int32
U32 = mybir.dt.uint32

N = 8192
H = 256
H2 = 2 * H
C_OUT = 40
K_TOP = 16
NCORES = 8
P = 128
ROWS_PER_CORE = N // NCORES          # 1024
CHUNKS = ROWS_PER_CORE // P          # 8
KT = H2 // P                         # 4 k-tiles of the 2H embedding dim
HKT = H // P                         # 2 k-tiles of the H dim
SIMW = 512                           # sim column chunk (one PSUM bank)
NCC = N // SIMW                      # 16 column chunks
GROUP = 2                            # row-tiles sharing one rhs stream pass
CCG = 4                              # column chunks per rhs load


def _split_waits(nc, maxw=1):
    """This container's walrus only accepts one sync-wait command per
    instruction; hoist excess waits onto preceding same-engine NOPs."""
    n_new = 0
    for bb in nc.main_func.blocks:
        new_insts = []
        for ins in bb.instructions:
            si = ins.sync_info
            if si is not None and si.on_wait and len(si.on_wait) > maxw:
                waits = list(si.on_wait)
                excess, keep = waits[:-maxw], waits[-maxw:]
                for i in range(0, len(excess), maxw):
                    nop = mybir.InstNoOp(
                        name=f"waitnop-{ins.name}-{i}",
                        engine=ins.engine,
                        ins=[],
                        outs=[],
                        sync_info=mybir.SyncInfo(
                            on_wait=excess[i:i + maxw], on_update=[]
                        ),
                    )
                    new_insts.append(nop)
                    n_new += 1
                si.on_wait = keep
            new_insts.append(ins)
        bb.instructions[:] = new_insts
    return n_new


def build(T, n=N, debug=False, repeat=1, phase_lim=9):
    """Build the SPMD program (identical on all cores; data differs).

    T: padded edge-slot count per destination row (host-computed max).
    """
    rows_per_core = n // NCORES
    chunks = rows_per_core // P
    ngroups = chunks // GROUP

    nc = bass.Bass(num_devices=NCORES)
    if debug:
        dbg_f1g = nc.dram_tensor("dbg_f1g", [n, H], F32, kind="ExternalOutput")
        dbg_agg = nc.dram_tensor("dbg_agg", [P, chunks * H], F32,
                                 kind="ExternalOutput")
        dbg_m16 = nc.dram_tensor("dbg_m16", [P, chunks * K_TOP], F32,
                                 kind="ExternalOutput")
        dbg_i16 = nc.dram_tensor("dbg_i16", [P, chunks * K_TOP], U32,
                                 kind="ExternalOutput")

    # ---- per-core inputs ----
    feat_d = nc.dram_tensor("feat", [rows_per_core, H], F32, kind="ExternalInput")
    # per-edge expanded features: [chunk][p][slot*H + h] = features[col of the
    # slot-th edge into local row chunk*128+p] (host-gathered; val=0 pads)
    fexp_d = nc.dram_tensor("fexp", [chunks, P, T * H], F32, kind="ExternalInput")
    eval_d = nc.dram_tensor("eval", [P, chunks * T], F32, kind="ExternalInput")
    # replicated small params (host pre-broadcast across partitions)
    cA_d = nc.dram_tensor("cA", [P, H], F32, kind="ExternalInput")
    cB_d = nc.dram_tensor("cB", [P, H], F32, kind="ExternalInput")
    cC_d = nc.dram_tensor("cC", [P, H], F32, kind="ExternalInput")
    bal_d = nc.dram_tensor("bal", [P, H2], F32, kind="ExternalInput")
    bias_d = nc.dram_tensor("bias", [P, C_OUT], F32, kind="ExternalInput")
    gcnw_d = nc.dram_tensor("gcnw", [H, C_OUT], F32, kind="ExternalInput")
    ident_d = nc.dram_tensor("ident", [P, P], F32, kind="ExternalInput")

    out_d = nc.dram_tensor("out", [rows_per_core, C_OUT], F32, kind="ExternalOutput")

    with tile.TileContext(nc) as tc:
        with tc.tile_pool(name="consts", bufs=1) as cp, \
             tc.tile_pool(name="persist", bufs=1) as pp, \
             tc.tile_pool(name="dram", bufs=1, space="DRAM") as dp:

            # ---- constants into SBUF ----
            cA = cp.tile([P, H], F32)
            cB = cp.tile([P, H], F32)
            cC = cp.tile([P, H], F32)
            bal = cp.tile([P, H2], F32)
            bias = cp.tile([P, C_OUT], F32)
            gcnw = cp.tile([P, HKT * C_OUT], F32)
            ident = cp.tile([P, P], F32)
            nc.sync.dma_start(out=cA[:], in_=cA_d[:])
            nc.sync.dma_start(out=cB[:], in_=cB_d[:])
            nc.sync.dma_start(out=cC[:], in_=cC_d[:])
            nc.sync.dma_start(out=bal[:], in_=bal_d[:])
            nc.sync.dma_start(out=bias[:], in_=bias_d[:])
            nc.sync.dma_start(
                out=gcnw[:].rearrange("p (k c) -> p k c", k=HKT),
                in_=gcnw_d[:].rearrange("(k p) c -> p k c", p=P))
            nc.sync.dma_start(out=ident[:], in_=ident_d[:])

            # ---- persistent SBUF tensors ----
            f1_all = pp.tile([P, chunks * H], F32)      # f1, row-chunk major
            agg_all = pp.tile([P, chunks * H], F32)     # A @ f1
            g2_all = pp.tile([P, chunks * H], F32)      # re_adj @ f1
            embTloc = pp.tile([P, KT * rows_per_core], F32)  # own embT (lhsT)
            m16 = pp.tile([P, chunks * K_TOP], F32)
            i16 = pp.tile([P, chunks * K_TOP], U32)
            v16 = pp.tile([P, chunks * K_TOP], F32)

            for rep in range(repeat):
                # ---- DRAM scratch (per rep: Shared tensors single-writer) ----
                f1loc = dp.tile([rows_per_core, H], F32, name=f"f1loc_{rep}")
                f1g = dp.tile([n, H], F32, addr_space="Shared",
                              name=f"f1g_{rep}")
                embT_loc_d = dp.tile([H2, rows_per_core], F32,
                                     name=f"embTl_{rep}")
                embT_g = dp.tile([NCORES * H2, rows_per_core], F32,
                                 addr_space="Shared", name=f"embTg_{rep}")

                # ===== P1: f1 for the local row block (batched) =====
                with tc.tile_pool(name=f"p1_{rep}", bufs=1) as p1:
                    W1 = chunks * H
                    ft = p1.tile([P, W1], F32)
                    nc.sync.dma_start(
                        out=ft[:].rearrange("p (r w) -> p r w", r=chunks),
                        in_=feat_d[:].rearrange("(r p) w -> p r w", p=P))
                    mB = lambda c: c[:, None, :].to_broadcast([P, chunks, H])
                    v3 = lambda t: t[:].rearrange("p (r w) -> p r w", r=chunks)
                    m = p1.tile([P, W1], F32)
                    nc.vector.tensor_tensor(out=v3(m), in0=v3(ft), in1=mB(cB),
                                            op=mybir.AluOpType.mult)
                    nc.vector.tensor_scalar(out=m[:], in0=m[:], scalar1=0.0,
                                            scalar2=None, op0=mybir.AluOpType.max)
                    nc.vector.tensor_tensor(out=v3(m), in0=v3(m), in1=mB(cA),
                                            op=mybir.AluOpType.mult)
                    v = p1.tile([P, W1], F32)
                    nc.vector.tensor_tensor(out=v3(v), in0=v3(ft), in1=mB(cC),
                                            op=mybir.AluOpType.mult)
                    nc.vector.tensor_tensor(out=m[:], in0=m[:], in1=v[:],
                                            op=mybir.AluOpType.add)
                    # elu(z) = (relu(z) - 1) + exp(min(z, 0))
                    nc.vector.tensor_scalar(out=v[:], in0=m[:], scalar1=0.0,
                                            scalar2=-1.0, op0=mybir.AluOpType.max,
                                            op1=mybir.AluOpType.add)
                    nc.vector.tensor_scalar(out=m[:], in0=m[:], scalar1=0.0,
                                            scalar2=None, op0=mybir.AluOpType.min)
                    nc.scalar.activation(out=m[:], in_=m[:],
                                         func=mybir.ActivationFunctionType.Exp)
                    nc.vector.tensor_tensor(out=f1_all[:], in0=v[:], in1=m[:],
                                            op=mybir.AluOpType.add)
                    nc.sync.dma_start(
                        out=f1loc[:].rearrange("(r p) w -> p r w", p=P),
                        in_=v3(f1_all))

                if phase_lim >= 2:
                    # ===== P1b: all-gather f1 =====
                    nc.gpsimd.collective_compute(
                        "AllGather", mybir.AluOpType.bypass,
                        replica_groups=[list(range(NCORES))],
                        ins=[f1loc.opt()], outs=[f1g.opt()],
                    )
                    if debug and rep == 0:
                        nc.sync.dma_start(out=dbg_f1g[:], in_=f1g[:])

                if phase_lim >= 3:
                    # ===== P2: segment-sum agg = A @ f1 =====
                    # f1 is elementwise, so compute it directly on the
                    # host-expanded per-edge features (no gather), scale by
                    # edge vals, reduce over slots.  12 instructions/chunk.
                    with tc.tile_pool(name=f"p2i_{rep}", bufs=1) as p2i, \
                         tc.tile_pool(name=f"p2x_{rep}", bufs=1) as p2x:
                        ev = p2i.tile([P, chunks * T], F32)
                        nc.sync.dma_start(out=ev[:], in_=eval_d[:])
                        xb = p2x.tile([P, T * H], F32)
                        vb = p2x.tile([P, T * H], F32)
                        eB = lambda c: c[:, None, :].to_broadcast([P, T, H])
                        x3 = xb[:].rearrange("p (s w) -> p s w", s=T)
                        v3e = vb[:].rearrange("p (s w) -> p s w", s=T)
                        for ci in range(chunks):
                            nc.sync.dma_start(out=xb[:], in_=fexp_d[ci])
                            nc.vector.tensor_tensor(out=v3e, in0=x3, in1=eB(cC),
                                                    op=mybir.AluOpType.mult)
                            nc.vector.tensor_tensor(out=x3, in0=x3, in1=eB(cB),
                                                    op=mybir.AluOpType.mult)
                            nc.vector.tensor_scalar(out=xb[:], in0=xb[:],
                                                    scalar1=0.0, scalar2=None,
                                                    op0=mybir.AluOpType.max)
                            nc.vector.tensor_tensor(out=x3, in0=x3, in1=eB(cA),
                                                    op=mybir.AluOpType.mult)
                            nc.vector.tensor_tensor(out=xb[:], in0=xb[:],
                                                    in1=vb[:],
                                                    op=mybir.AluOpType.add)
                            # elu(z) = (relu(z) - 1) + exp(min(z, 0))
                            nc.vector.tensor_scalar(out=vb[:], in0=xb[:],
                                                    scalar1=0.0, scalar2=-1.0,
                                                    op0=mybir.AluOpType.max,
                                                    op1=mybir.AluOpType.add)
                            nc.vector.tensor_scalar(out=xb[:], in0=xb[:],
                                                    scalar1=0.0, scalar2=None,
                                                    op0=mybir.AluOpType.min)
                            nc.scalar.activation(
                                out=xb[:], in_=xb[:],
                                func=mybir.ActivationFunctionType.Exp)
                            nc.vector.tensor_tensor(out=xb[:], in0=xb[:],
                                                    in1=vb[:],
                                                    op=mybir.AluOpType.add)
                            nc.vector.tensor_tensor(
                                out=x3, in0=x3,
                                in1=ev[:, ci * T:(ci + 1) * T, None]
                                    .to_broadcast([P, T, H]),
                                op=mybir.AluOpType.mult)
                            nc.vector.tensor_reduce(
                                out=agg_all[:, ci * H:(ci + 1) * H],
                                in_=xb[:].rearrange("p (s w) -> p w s", s=T),
                                axis=mybir.AxisListType.X,
                                op=mybir.AluOpType.add)
                    if debug and rep == 0:
                        nc.sync.dma_start(out=dbg_agg[:], in_=agg_all[:])

                if phase_lim >= 4:
                    # ===== P3: embedding build + transpose + all-gather =====
                    with tc.tile_pool(name=f"p3_{rep}", bufs=1) as p3, \
                         tc.tile_pool(name=f"p3ps_{rep}", bufs=2,
                                      space="PSUM") as p3ps:
                        W3 = chunks * H2
                        zc = p3.tile([P, W3], F32)
                        zc3 = zc[:].rearrange("p (r w) -> p r w", r=chunks)
                        nc.vector.tensor_tensor(
                            out=zc3[:, :, 0:H],
                            in0=f1_all[:].rearrange("p (r w) -> p r w", r=chunks),
                            in1=bal[:, None, 0:H].to_broadcast([P, chunks, H]),
                            op=mybir.AluOpType.mult)
                        nc.vector.tensor_tensor(
                            out=zc3[:, :, H:H2],
                            in0=agg_all[:].rearrange("p (r w) -> p r w", r=chunks),
                            in1=bal[:, None, H:H2].to_broadcast([P, chunks, H]),
                            op=mybir.AluOpType.mult)
                        sq = p3.tile([P, W3], F32)
                        nc.vector.tensor_tensor(out=sq[:], in0=zc[:], in1=zc[:],
                                                op=mybir.AluOpType.mult)
                        n2 = p3.tile([P, chunks], F32)
                        nc.vector.tensor_reduce(
                            out=n2[:],
                            in_=sq[:].rearrange("p (r w) -> p r w", r=chunks),
                            axis=mybir.AxisListType.X,
                            op=mybir.AluOpType.add)
                        nc.scalar.activation(out=n2[:], in_=n2[:],
                                             func=mybir.ActivationFunctionType.Sqrt)
                        inv = p3.tile([P, chunks], F32)
                        nc.vector.reciprocal(out=inv[:], in_=n2[:])
                        nc.vector.tensor_tensor(
                            out=zc3, in0=zc3,
                            in1=inv[:, :, None].to_broadcast([P, chunks, H2]),
                            op=mybir.AluOpType.mult)
                        # transpose all chunks: [128, rt*512 + k*128] blocks
                        for half in range(2):
                            psT = p3ps.tile([P, 4 * KT * P], F32, space="PSUM",
                                            tag="psT")
                            for j in range(4 * KT):
                                rt = half * 4 + j // KT
                                k = j % KT
                                nc.tensor.transpose(
                                    out=psT[:, j * P:(j + 1) * P],
                                    in_=zc[:, rt * H2 + k * P:
                                           rt * H2 + (k + 1) * P],
                                    identity=ident[:])
                            # embTloc layout: [p, k*rows + rt*128 + r]
                            nc.scalar.copy(
                                out=embTloc[:]
                                .rearrange("p (k r) -> p k r", k=KT)
                                [:, :, half * 4 * P:(half + 1) * 4 * P]
                                .rearrange("p k (rt r) -> p rt k r", rt=4),
                                in_=psT[:].rearrange("p (rt k r) -> p rt k r",
                                                     rt=4, k=KT))
                        # one DMA: embT_loc_d[k*128+p, r] = embTloc[p, k*rows+r]
                        nc.sync.dma_start(
                            out=embT_loc_d[:].rearrange("(k p) r -> p k r", p=P),
                            in_=embTloc[:].rearrange("p (k r) -> p k r", k=KT))

                    # ===== P3b: all-gather transposed embedding =====
                    nc.gpsimd.collective_compute(
                        "AllGather", mybir.AluOpType.bypass,
                        replica_groups=[list(range(NCORES))],
                        ins=[embT_loc_d.opt()], outs=[embT_g.opt()],
                    )

                if phase_lim >= 5:
                    # ===== P4: sim row-block + top-k + gather-combine =====
                    with tc.tile_pool(name=f"p4rhs_{rep}", bufs=2) as p4rhs, \
                         tc.tile_pool(name=f"p4sim_{rep}", bufs=GROUP) as p4sim, \
                         tc.tile_pool(name=f"p4g_{rep}", bufs=2) as p4g, \
                         tc.tile_pool(name=f"p4ps_{rep}", bufs=2,
                                      space="PSUM") as p4ps:
                        for g in range(ngroups):
                            rts = [g * GROUP + j for j in range(GROUP)]
                            sims = {rt: p4sim.tile([P, n], F32, tag="simbuf",
                                                   name=f"sim_rt{rt}")
                                    for rt in rts}
                            for cg in range(NCC // CCG):
                                # rhs covers 4 cc = 2048 global cols, all k.
                                # global col block c2 spans core (2cg+c2)'s rows
                                rh = p4rhs.tile([P, KT * CCG * SIMW], F32,
                                                tag="rhs")
                                rh4 = rh[:].rearrange(
                                    "p (k c2 w) -> p k c2 w", k=KT, c2=2)
                                for c2 in range(2):
                                    blk = (2 * cg + c2) * H2
                                    nc.sync.dma_start(
                                        out=rh4[:, :, c2, :],
                                        in_=embT_g[blk:blk + H2, :].rearrange(
                                            "(k p) r -> p k r", p=P))
                                for rt in rts:
                                    pss = p4ps.tile([P, CCG * SIMW], F32,
                                                    space="PSUM", tag="pss")
                                    for q in range(CCG):
                                        for k in range(KT):
                                            nc.tensor.matmul(
                                                out=pss[:, q * SIMW:
                                                        (q + 1) * SIMW],
                                                lhsT=embTloc[
                                                    :, k * rows_per_core
                                                    + rt * P:
                                                    k * rows_per_core
                                                    + (rt + 1) * P],
                                                rhs=rh[:, (k * CCG + q) * SIMW:
                                                       (k * CCG + q + 1) * SIMW],
                                                start=(k == 0),
                                                stop=(k == KT - 1))
                                    nc.scalar.copy(
                                        out=sims[rt][:, cg * CCG * SIMW:
                                                     (cg + 1) * CCG * SIMW],
                                        in_=pss[:])
                            if phase_lim < 6:
                                for rt in rts:
                                    mx = p4g.tile([P, 8], F32, tag="mx")
                                    nc.vector.max(out=mx[:], in_=sims[rt][:])
                                continue
                            for rt in rts:
                                sim = sims[rt]
                                sl16 = slice(rt * K_TOP, (rt + 1) * K_TOP)
                                sl8a = slice(rt * K_TOP, rt * K_TOP + 8)
                                sl8b = slice(rt * K_TOP + 8, (rt + 1) * K_TOP)
                                nc.vector.max(out=m16[:, sl8a], in_=sim[:])
                                nc.vector.max_index(out=i16[:, sl8a],
                                                    in_max=m16[:, sl8a],
                                                    in_values=sim[:])
                                nc.vector.match_replace(out=sim[:],
                                                        in_to_replace=m16[:, sl8a],
                                                        in_values=sim[:],
                                                        imm_value=-1e30)
                                nc.vector.max(out=m16[:, sl8b], in_=sim[:])
                                nc.vector.max_index(out=i16[:, sl8b],
                                                    in_max=m16[:, sl8b],
                                                    in_values=sim[:])
                                # P5: g2 = sum_j relu(v_j) * f1[idx_j]
                                nc.vector.tensor_scalar(
                                    out=v16[:, sl16], in0=m16[:, sl16],
                                    scalar1=0.0, scalar2=None,
                                    op0=mybir.AluOpType.max)
                                g5 = p4g.tile([P, K_TOP * H], F32, tag="g5",
                                              name=f"g5_{rt}")
                                for j in range(K_TOP):
                                    nc.gpsimd.indirect_dma_start(
                                        out=g5[:, j * H:(j + 1) * H],
                                        out_offset=None,
                                        in_=f1g[:, :],
                                        in_offset=bass.IndirectOffsetOnAxis(
                                            ap=i16[:, rt * K_TOP + j:
                                                   rt * K_TOP + j + 1], axis=0),
                                    )
                                nc.vector.tensor_tensor(
                                    out=g5[:].rearrange("p (t w) -> p t w",
                                                        t=K_TOP),
                                    in0=g5[:].rearrange("p (t w) -> p t w",
                                                        t=K_TOP),
                                    in1=v16[:, sl16, None]
                                        .to_broadcast([P, K_TOP, H]),
                                    op=mybir.AluOpType.mult)
                                nc.vector.tensor_reduce(
                                    out=g2_all[:, rt * H:(rt + 1) * H],
                                    in_=g5[:].rearrange("p (t w) -> p w t",
                                                        t=K_TOP),
                                    axis=mybir.AxisListType.X,
                                    op=mybir.AluOpType.add)
                        if debug and rep == 0 and phase_lim >= 6:
                            nc.sync.dma_start(out=dbg_m16[:], in_=m16[:])
                            nc.sync.dma_start(out=dbg_i16[:], in_=i16[:])

                if phase_lim >= 6:
                    # ===== P6: out = relu(0.5*(agg+g2) @ W + b) =====
                    # (0.5 is folded into W host-side)
                    with tc.tile_pool(name=f"p6_{rep}", bufs=1) as p6, \
                         tc.tile_pool(name=f"p6ps_{rep}", bufs=1,
                                      space="PSUM") as p6ps:
                        zf = p6.tile([P, chunks * H], F32)
                        nc.vector.tensor_tensor(out=zf[:], in0=agg_all[:],
                                                in1=g2_all[:],
                                                op=mybir.AluOpType.add)
                        psT = p6ps.tile([P, chunks * HKT * P], F32,
                                        space="PSUM", name="psTz")
                        for j in range(chunks * HKT):
                            nc.tensor.transpose(
                                out=psT[:, j * P:(j + 1) * P],
                                in_=zf[:, j * P:(j + 1) * P],
                                identity=ident[:])
                        zT = p6.tile([P, chunks * HKT * P], F32)
                        nc.scalar.copy(out=zT[:], in_=psT[:])
                        psy = p6ps.tile([P, chunks * C_OUT], F32,
                                        space="PSUM", name="psy")
                        for rt in range(chunks):
                            for k in range(HKT):
                                nc.tensor.matmul(
                                    out=psy[:, rt * C_OUT:(rt + 1) * C_OUT],
                                    lhsT=zT[:, (rt * HKT + k) * P:
                                            (rt * HKT + k + 1) * P],
                                    rhs=gcnw[:, k * C_OUT:(k + 1) * C_OUT],
                                    start=(k == 0), stop=(k == HKT - 1))
                        ot = p6.tile([P, chunks * C_OUT], F32)
                        nc.scalar.copy(out=ot[:], in_=psy[:])
                        o3 = ot[:].rearrange("p (r w) -> p r w", r=chunks)
                        nc.vector.tensor_tensor(
                            out=o3, in0=o3,
                            in1=bias[:, None, :].to_broadcast([P, chunks, C_OUT]),
                            op=mybir.AluOpType.add)
                        nc.vector.tensor_scalar(out=ot[:], in0=ot[:],
                                                scalar1=0.0, scalar2=None,
                                                op0=mybir.AluOpType.max)
                        nc.sync.dma_start(
                            out=out_d[:].rearrange("(r p) w -> p r w", p=P),
                            in_=o3)

            if phase_lim < 6:
                with tc.tile_pool(name="dummyout", bufs=1) as dop:
                    zz = dop.tile([P, C_OUT], F32)
                    nc.vector.memset(zz[:], 0.0)
                    for rt in range(chunks):
                        nc.sync.dma_start(out=out_d[rt * P:(rt + 1) * P, :],
                                          in_=zz[:])

    return nc


def prep_inputs(features, adj_rows, adj_cols, adj_vals, tokens, wp_weight,
                global_token, pre_token_w, combine_w, balance_w, gcn_w, gcn_b,
                n=N):
    """Host-side sharding: row-block features, pad each destination row's
    edge list to T slots, pre-broadcast the small parameters."""
    rows_per_core = n // NCORES
    chunks = rows_per_core // P

    features = np.ascontiguousarray(np.asarray(features, dtype=np.float32))
    r = np.asarray(adj_rows).astype(np.int64)
    c = np.asarray(adj_cols).astype(np.int64)
    v = np.asarray(adj_vals, dtype=np.float32)
    E = r.shape[0]

    pre_token = (np.asarray(wp_weight, np.float32) @
                 np.asarray(tokens, np.float32)).reshape(-1)       # [H]
    cw = np.asarray(combine_w, np.float32).reshape(-1)
    cA = (cw[0] * np.asarray(global_token, np.float32)).reshape(-1)
    cB = pre_token
    cC = (cw[1] * np.asarray(pre_token_w, np.float32)).reshape(-1)
    bal = np.asarray(balance_w, np.float32).reshape(-1)
    bias = np.asarray(gcn_b, np.float32).reshape(-1)

    bcast = lambda x: np.ascontiguousarray(np.tile(x[None, :], (P, 1)))
    cA_b, cB_b, cC_b = bcast(cA), bcast(cB), bcast(cC)
    bal_b, bias_b = bcast(bal), bcast(bias)
    gcnw = np.ascontiguousarray(0.5 * np.asarray(gcn_w, np.float32))
    ident = np.eye(P, dtype=np.float32)

    # pad each destination row's edges to T slots
    order = np.argsort(r, kind="stable")
    rs, cs, vs = r[order], c[order], v[order]
    cnt = np.bincount(rs, minlength=n)
    T = int(np.ceil(max(1, cnt.max()) / 8) * 8)
    starts = np.concatenate([[0], np.cumsum(cnt)])
    slot = np.arange(E) - starts[rs]
    ecol = np.zeros((n, T), np.int32)
    evalv = np.zeros((n, T), np.float32)
    ecol[rs, slot] = cs.astype(np.int32)
    evalv[rs, slot] = vs
    # expanded per-edge features: fexp[row, s, :] = features[ecol[row, s]]
    fexp = features[ecol]                      # [n, T, H] (val=0 kills pads)
    fexp = np.ascontiguousarray(
        fexp.reshape(NCORES, chunks, P, T * H))
    # eval SBUF layout per core: [p, ci*T + s] for local row ci*128+p
    evalv = evalv.reshape(NCORES, chunks, P, T).transpose(0, 2, 1, 3)
    evalv = np.ascontiguousarray(evalv.reshape(NCORES, P, chunks * T))

    in_maps = []
    for core in range(NCORES):
        in_maps.append({
            "feat": features[core * rows_per_core:(core + 1) * rows_per_core],
            "fexp": fexp[core], "eval": evalv[core],
            "cA": cA_b, "cB": cB_b, "cC": cC_b, "bal": bal_b, "bias": bias_b,
            "gcnw": gcnw, "ident": ident,
        })
    return in_maps, T


_BUILD_CACHE = {}


def kernel(features, adj_rows, adj_cols, adj_vals, down_k,
           tokens, wp_weight, global_token, pre_token_w, combine_w,
           balance_w, gcn_w, gcn_b):
    k = int(np.asarray(down_k))
    assert k == K_TOP, f"kernel hardcodes top-k={K_TOP}, got {k}"
    in_maps, T = prep_inputs(features, adj_rows, adj_cols, adj_vals, tokens,
                             wp_weight, global_token, pre_token_w, combine_w,
                             balance_w, gcn_w, gcn_b)
    if T not in _BUILD_CACHE:
        nc_new = build(T)
        _split_waits(nc_new)   # hardware-only fixup; breaks CoreSim if applied
        _BUILD_CACHE[T] = nc_new
    nc = _BUILD_CACHE[T]
    res = run_bass_kernel_spmd(nc, in_maps, list(range(NCORES)))
    out = np.concatenate([res.results[i]["out"] for i in range(NCORES)], axis=0)
    return out.astype(np.float32)


# revision 10
# speedup vs baseline: 120.1278x; 2.5019x over previous
"""Trainium2 Bass kernel for nn_DownModel (GNN message passing + kNN graph + GCN).

Math (from the reference):
  f1   = elu(c00*global*relu(pre_token*feat) + c01*pre_token_w*feat)      [N,H]
  agg  = scatter-add over E edges of adj_vals * f1[adj_cols]              [N,H]
  emb  = normalize_rows(balance_w * [f1 | agg])                           [N,2H]
  sim  = emb @ emb.T ; vals,idx = top_k(sim, 16)                          [N,N]
  g2   = sum_j relu(vals_j) * f1[idx_j]                                   [N,H]
  out  = relu(0.5*(agg + g2) @ gcn_w + b)                                 [N,C]
  (uses A@y = (A@f1)@W and re@y = (re@f1)@W to defer the GCN matmul)

This environment is instruction-dispatch-bound (~25-90us per instruction
regardless of size), so the design minimizes instruction count:
  - segment-sum = one 64-index gather + scale + strided tensor_reduce per
    128-row chunk (edges padded per *row* to T slots host-side)
  - top-k gather = one 16-index gather per row-tile
  - sim row-block matmul in f32 (low precision provably flips top-16
    membership and blows the tolerance)
"""

import numpy as np

import concourse.bass as bass
import concourse.mybir as mybir
import concourse.tile as tile
from concourse.bass_utils import run_bass_kernel_spmd

F32 = mybir.dt.float32
I32 = mybir.dt.int32
U32 = mybir.dt.uint32

N = 8192
H = 256
H2 = 2 * H
C_OUT = 40
K_TOP = 16
NCORES = 8
P = 128
ROWS_PER_CORE = N // NCORES          # 1024
CHUNKS = ROWS_PER_CORE // P          # 8
KT = H2 // P                         # 4 k-tiles of the 2H embedding dim
HKT = H // P                         # 2 k-tiles of the H dim
SIMW = 512                           # sim column chunk (one PSUM bank)
NCC = N // SIMW                      # 16 column chunks
GROUP = 2                            # row-tiles sharing one rhs stream pass
CCG = 4                              # column chunks per rhs load


def _split_waits(nc, maxw=1):
    """This container's walrus only accepts one sync-wait command per
    instruction; hoist excess waits onto preceding same-engine NOPs."""
    n_new = 0
    for bb in nc.main_func.blocks:
        new_insts = []
        for ins in bb.instructions:
            si = ins.sync_info
            if si is not None and si.on_wait and len(si.on_wait) > maxw:
                waits = list(si.on_wait)
                excess, keep = waits[:-maxw], waits[-maxw:]
                for i in range(0, len(excess), maxw):
                    nop = mybir.InstNoOp(
                        name=f"waitnop-{ins.name}-{i}",
                        engine=ins.engine,
                        ins=[],
                        outs=[],
                        sync_info=mybir.SyncInfo(
                            on_wait=excess[i:i + maxw], on_update=[]
                        ),
                    )
                    new_insts.append(nop)
                    n_new += 1
                si.on_wait = keep
            new_insts.append(ins)
        bb.instructions[:] = new_insts
    return n_new


def build(T, n=N, debug=False, repeat=1, phase_lim=9):
    """Build the SPMD program (identical on all cores; data differs).

    T: padded edge-slot count per destination row (host-computed max).
    """
    rows_per_core = n // NCORES
    chunks = rows_per_core // P
    ngroups = chunks // GROUP

    nc = bass.Bass(num_devices=NCORES)
    if debug:
        dbg_f1g = nc.dram_tensor("dbg_f1g", [n, H], F32, kind="ExternalOutput")
        dbg_agg = nc.dram_tensor("dbg_agg", [P, chunks * H], F32,
                                 kind="ExternalOutput")
        dbg_m16 = nc.dram_tensor("dbg_m16", [P, chunks * K_TOP], F32,
                                 kind="ExternalOutput")
        dbg_i16 = nc.dram_tensor("dbg_i16", [P, chunks * K_TOP], U32,
                                 kind="ExternalOutput")

    # ---- per-core inputs ----
    feat_d = nc.dram_tensor("feat", [rows_per_core, H], F32, kind="ExternalInput")
    # dense adjacency blocks for this core's rows, transposed per k-block:
    # ablk[ci, k*128+s, d] = A[core*1024 + ci*128 + d, k*128 + s]
    nkb = n // P
    ablk_d = nc.dram_tensor("ablk", [chunks, nkb * P, P], F32,
                            kind="ExternalInput")
    # replicated small params (host pre-broadcast across partitions)
    cA_d = nc.dram_tensor("cA", [P, H], F32, kind="ExternalInput")
    cB_d = nc.dram_tensor("cB", [P, H], F32, kind="ExternalInput")
    cC_d = nc.dram_tensor("cC", [P, H], F32, kind="ExternalInput")
    bal_d = nc.dram_tensor("bal", [P, H2], F32, kind="ExternalInput")
    bias_d = nc.dram_tensor("bias", [P, C_OUT], F32, kind="ExternalInput")
    gcnw_d = nc.dram_tensor("gcnw", [H, C_OUT], F32, kind="ExternalInput")
    ident_d = nc.dram_tensor("ident", [P, P], F32, kind="ExternalInput")

    out_d = nc.dram_tensor("out", [rows_per_core, C_OUT], F32, kind="ExternalOutput")

    with tile.TileContext(nc) as tc:
        with tc.tile_pool(name="consts", bufs=1) as cp, \
             tc.tile_pool(name="persist", bufs=1) as pp, \
             tc.tile_pool(name="dram", bufs=1, space="DRAM") as dp:

            # ---- constants into SBUF ----
            cA = cp.tile([P, H], F32)
            cB = cp.tile([P, H], F32)
            cC = cp.tile([P, H], F32)
            bal = cp.tile([P, H2], F32)
            bias = cp.tile([P, C_OUT], F32)
            gcnw = cp.tile([P, HKT * C_OUT], F32)
            ident = cp.tile([P, P], F32)
            nc.sync.dma_start(out=cA[:], in_=cA_d[:])
            nc.sync.dma_start(out=cB[:], in_=cB_d[:])
            nc.sync.dma_start(out=cC[:], in_=cC_d[:])
            nc.sync.dma_start(out=bal[:], in_=bal_d[:])
            nc.sync.dma_start(out=bias[:], in_=bias_d[:])
            nc.sync.dma_start(
                out=gcnw[:].rearrange("p (k c) -> p k c", k=HKT),
                in_=gcnw_d[:].rearrange("(k p) c -> p k c", p=P))
            nc.sync.dma_start(out=ident[:], in_=ident_d[:])

            # ---- persistent SBUF tensors ----
            f1_all = pp.tile([P, chunks * H], F32)      # f1, row-chunk major
            agg_all = pp.tile([P, chunks * H], F32)     # A @ f1
            g2_all = pp.tile([P, chunks * H], F32)      # re_adj @ f1
            embTloc = pp.tile([P, KT * rows_per_core], F32)  # own embT (lhsT)
            m16 = pp.tile([P, chunks * K_TOP], F32)
            i16 = pp.tile([P, chunks * K_TOP], U32)
            v16 = pp.tile([P, chunks * K_TOP], F32)

            for rep in range(repeat):
                # ---- DRAM scratch (per rep: Shared tensors single-writer) ----
                f1loc = dp.tile([rows_per_core, H], F32, name=f"f1loc_{rep}")
                f1g = dp.tile([n, H], F32, addr_space="Shared",
                              name=f"f1g_{rep}")
                embT_loc_d = dp.tile([H2, rows_per_core], F32,
                                     name=f"embTl_{rep}")
                embT_g = dp.tile([NCORES * H2, rows_per_core], F32,
                                 addr_space="Shared", name=f"embTg_{rep}")

                # ===== P1: f1 for the local row block (batched) =====
                with tc.tile_pool(name=f"p1_{rep}", bufs=1) as p1:
                    W1 = chunks * H
                    ft = p1.tile([P, W1], F32)
                    nc.sync.dma_start(
                        out=ft[:].rearrange("p (r w) -> p r w", r=chunks),
                        in_=feat_d[:].rearrange("(r p) w -> p r w", p=P))
                    mB = lambda c: c[:, None, :].to_broadcast([P, chunks, H])
                    v3 = lambda t: t[:].rearrange("p (r w) -> p r w", r=chunks)
                    m = p1.tile([P, W1], F32)
                    nc.vector.tensor_tensor(out=v3(m), in0=v3(ft), in1=mB(cB),
                                            op=mybir.AluOpType.mult)
                    nc.vector.tensor_scalar(out=m[:], in0=m[:], scalar1=0.0,
                                            scalar2=None, op0=mybir.AluOpType.max)
                    nc.vector.tensor_tensor(out=v3(m), in0=v3(m), in1=mB(cA),
                                            op=mybir.AluOpType.mult)
                    v = p1.tile([P, W1], F32)
                    nc.vector.tensor_tensor(out=v3(v), in0=v3(ft), in1=mB(cC),
                                            op=mybir.AluOpType.mult)
                    nc.vector.tensor_tensor(out=m[:], in0=m[:], in1=v[:],
                                            op=mybir.AluOpType.add)
                    # elu(z) = (relu(z) - 1) + exp(min(z, 0))
                    nc.vector.tensor_scalar(out=v[:], in0=m[:], scalar1=0.0,
                                            scalar2=-1.0, op0=mybir.AluOpType.max,
                                            op1=mybir.AluOpType.add)
                    nc.vector.tensor_scalar(out=m[:], in0=m[:], scalar1=0.0,
                                            scalar2=None, op0=mybir.AluOpType.min)
                    nc.scalar.activation(out=m[:], in_=m[:],
                                         func=mybir.ActivationFunctionType.Exp)
                    nc.vector.tensor_tensor(out=f1_all[:], in0=v[:], in1=m[:],
                                            op=mybir.AluOpType.add)
                    nc.sync.dma_start(
                        out=f1loc[:].rearrange("(r p) w -> p r w", p=P),
                        in_=v3(f1_all))

                if phase_lim >= 2:
                    # ===== P1b: all-gather f1 =====
                    nc.gpsimd.collective_compute(
                        "AllGather", mybir.AluOpType.bypass,
                        replica_groups=[list(range(NCORES))],
                        ins=[f1loc.opt()], outs=[f1g.opt()],
                    )
                    if debug and rep == 0:
                        nc.sync.dma_start(out=dbg_f1g[:], in_=f1g[:])

                if phase_lim >= 3:
                    # ===== P2: segment-sum agg = A @ f1, dense-blocked =====
                    # lhsT = host-built A^T [128 src, 128 dst] blocks, rhs =
                    # gathered f1 k-blocks; 64-deep PSUM chain per chunk.
                    with tc.tile_pool(name=f"p2f_{rep}", bufs=1) as p2f, \
                         tc.tile_pool(name=f"p2a_{rep}", bufs=2) as p2a, \
                         tc.tile_pool(name=f"p2ps_{rep}", bufs=2,
                                      space="PSUM") as p2ps:
                        f1r = p2f.tile([P, nkb * H], F32)
                        nc.sync.dma_start(
                            out=f1r[:].rearrange("p (k w) -> p k w", k=nkb),
                            in_=f1g[:].rearrange("(k p) w -> p k w", p=P))
                        for ci in range(chunks):
                            at = p2a.tile([P, nkb * P], F32, tag="at",
                                          name=f"at_{ci}")
                            nc.sync.dma_start(
                                out=at[:].rearrange("p (k d) -> p k d", k=nkb),
                                in_=ablk_d[ci].rearrange("(k p) d -> p k d",
                                                         p=P))
                            psa = p2ps.tile([P, H], F32, space="PSUM",
                                            tag="psa")
                            for k in range(nkb):
                                nc.tensor.matmul(
                                    out=psa[:],
                                    lhsT=at[:, k * P:(k + 1) * P],
                                    rhs=f1r[:, k * H:(k + 1) * H],
                                    start=(k == 0), stop=(k == nkb - 1))
                            nc.scalar.copy(out=agg_all[:, ci * H:(ci + 1) * H],
                                           in_=psa[:])
                    if debug and rep == 0:
                        nc.sync.dma_start(out=dbg_agg[:], in_=agg_all[:])

                if phase_lim >= 4:
                    # ===== P3: embedding build + transpose + all-gather =====
                    with tc.tile_pool(name=f"p3_{rep}", bufs=1) as p3, \
                         tc.tile_pool(name=f"p3ps_{rep}", bufs=2,
                                      space="PSUM") as p3ps:
                        W3 = chunks * H2
                        zc = p3.tile([P, W3], F32)
                        zc3 = zc[:].rearrange("p (r w) -> p r w", r=chunks)
                        nc.vector.tensor_tensor(
                            out=zc3[:, :, 0:H],
                            in0=f1_all[:].rearrange("p (r w) -> p r w", r=chunks),
                            in1=bal[:, None, 0:H].to_broadcast([P, chunks, H]),
                            op=mybir.AluOpType.mult)
                        nc.vector.tensor_tensor(
                            out=zc3[:, :, H:H2],
                            in0=agg_all[:].rearrange("p (r w) -> p r w", r=chunks),
                            in1=bal[:, None, H:H2].to_broadcast([P, chunks, H]),
                            op=mybir.AluOpType.mult)
                        sq = p3.tile([P, W3], F32)
                        nc.vector.tensor_tensor(out=sq[:], in0=zc[:], in1=zc[:],
                                                op=mybir.AluOpType.mult)
                        n2 = p3.tile([P, chunks], F32)
                        nc.vector.tensor_reduce(
                            out=n2[:],
                            in_=sq[:].rearrange("p (r w) -> p r w", r=chunks),
                            axis=mybir.AxisListType.X,
                            op=mybir.AluOpType.add)
                        nc.scalar.activation(out=n2[:], in_=n2[:],
                                             func=mybir.ActivationFunctionType.Sqrt)
                        inv = p3.tile([P, chunks], F32)
                        nc.vector.reciprocal(out=inv[:], in_=n2[:])
                        nc.vector.tensor_tensor(
                            out=zc3, in0=zc3,
                            in1=inv[:, :, None].to_broadcast([P, chunks, H2]),
                            op=mybir.AluOpType.mult)
                        # transpose all chunks: [128, rt*512 + k*128] blocks
                        for half in range(2):
                            psT = p3ps.tile([P, 4 * KT * P], F32, space="PSUM",
                                            tag="psT")
                            for j in range(4 * KT):
                                rt = half * 4 + j // KT
                                k = j % KT
                                nc.tensor.transpose(
                                    out=psT[:, j * P:(j + 1) * P],
                                    in_=zc[:, rt * H2 + k * P:
                                           rt * H2 + (k + 1) * P],
                                    identity=ident[:])
                            # embTloc layout: [p, k*rows + rt*128 + r]
                            nc.scalar.copy(
                                out=embTloc[:]
                                .rearrange("p (k r) -> p k r", k=KT)
                                [:, :, half * 4 * P:(half + 1) * 4 * P]
                                .rearrange("p k (rt r) -> p rt k r", rt=4),
                                in_=psT[:].rearrange("p (rt k r) -> p rt k r",
                                                     rt=4, k=KT))
                        # one DMA: embT_loc_d[k*128+p, r] = embTloc[p, k*rows+r]
                        nc.sync.dma_start(
                            out=embT_loc_d[:].rearrange("(k p) r -> p k r", p=P),
                            in_=embTloc[:].rearrange("p (k r) -> p k r", k=KT))

                    # ===== P3b: all-gather transposed embedding =====
                    nc.gpsimd.collective_compute(
                        "AllGather", mybir.AluOpType.bypass,
                        replica_groups=[list(range(NCORES))],
                        ins=[embT_loc_d.opt()], outs=[embT_g.opt()],
                    )

                if phase_lim >= 5:
                    # ===== P4: sim row-block + top-k + gather-combine =====
                    with tc.tile_pool(name=f"p4rhs_{rep}", bufs=2) as p4rhs, \
                         tc.tile_pool(name=f"p4sim_{rep}", bufs=GROUP) as p4sim, \
                         tc.tile_pool(name=f"p4g_{rep}", bufs=2) as p4g, \
                         tc.tile_pool(name=f"p4ps_{rep}", bufs=2,
                                      space="PSUM") as p4ps:
                        for g in range(ngroups):
                            rts = [g * GROUP + j for j in range(GROUP)]
                            sims = {rt: p4sim.tile([P, n], F32, tag="simbuf",
                                                   name=f"sim_rt{rt}")
                                    for rt in rts}
                            for cg in range(NCC // CCG):
                                # rhs covers 4 cc = 2048 global cols, all k.
                                # global col block c2 spans core (2cg+c2)'s rows
                                rh = p4rhs.tile([P, KT * CCG * SIMW], F32,
                                                tag="rhs")
                                rh4 = rh[:].rearrange(
                                    "p (k c2 w) -> p k c2 w", k=KT, c2=2)
                                for c2 in range(2):
                                    blk = (2 * cg + c2) * H2
                                    nc.sync.dma_start(
                                        out=rh4[:, :, c2, :],
                                        in_=embT_g[blk:blk + H2, :].rearrange(
                                            "(k p) r -> p k r", p=P))
                                for rt in rts:
                                    pss = p4ps.tile([P, CCG * SIMW], F32,
                                                    space="PSUM", tag="pss")
                                    for q in range(CCG):
                                        for k in range(KT):
                                            nc.tensor.matmul(
                                                out=pss[:, q * SIMW:
                                                        (q + 1) * SIMW],
                                                lhsT=embTloc[
                                                    :, k * rows_per_core
                                                    + rt * P:
                                                    k * rows_per_core
                                                    + (rt + 1) * P],
                                                rhs=rh[:, (k * CCG + q) * SIMW:
                                                       (k * CCG + q + 1) * SIMW],
                                                start=(k == 0),
                                                stop=(k == KT - 1))
                                    nc.scalar.copy(
                                        out=sims[rt][:, cg * CCG * SIMW:
                                                     (cg + 1) * CCG * SIMW],
                                        in_=pss[:])
                            if phase_lim < 6:
                                for rt in rts:
                                    mx = p4g.tile([P, 8], F32, tag="mx")
                                    nc.vector.max(out=mx[:], in_=sims[rt][:])
                                continue
                            for rt in rts:
                                sim = sims[rt]
                                sl16 = slice(rt * K_TOP, (rt + 1) * K_TOP)
                                sl8a = slice(rt * K_TOP, rt * K_TOP + 8)
                                sl8b = slice(rt * K_TOP + 8, (rt + 1) * K_TOP)
                                nc.vector.max(out=m16[:, sl8a], in_=sim[:])
                                nc.vector.max_index(out=i16[:, sl8a],
                                                    in_max=m16[:, sl8a],
                                                    in_values=sim[:])
                                nc.vector.match_replace(out=sim[:],
                                                        in_to_replace=m16[:, sl8a],
                                                        in_values=sim[:],
                                                        imm_value=-1e30)
                                nc.vector.max(out=m16[:, sl8b], in_=sim[:])
                                nc.vector.max_index(out=i16[:, sl8b],
                                                    in_max=m16[:, sl8b],
                                                    in_values=sim[:])
                                # P5: g2 = sum_j relu(v_j) * f1[idx_j]
                                nc.vector.tensor_scalar(
                                    out=v16[:, sl16], in0=m16[:, sl16],
                                    scalar1=0.0, scalar2=None,
                                    op0=mybir.AluOpType.max)
                                g5 = p4g.tile([P, K_TOP * H], F32, tag="g5",
                                              name=f"g5_{rt}")
                                for j in range(K_TOP):
                                    nc.gpsimd.indirect_dma_start(
                                        out=g5[:, j * H:(j + 1) * H],
                                        out_offset=None,
                                        in_=f1g[:, :],
                                        in_offset=bass.IndirectOffsetOnAxis(
                                            ap=i16[:, rt * K_TOP + j:
                                                   rt * K_TOP + j + 1], axis=0),
                                    )
                                nc.vector.tensor_tensor(
                                    out=g5[:].rearrange("p (t w) -> p t w",
                                                        t=K_TOP),
                                    in0=g5[:].rearrange("p (t w) -> p t w",
                                                        t=K_TOP),
                                    in1=v16[:, sl16, None]
                                        .to_broadcast([P, K_TOP, H]),
                                    op=mybir.AluOpType.mult)
                                nc.vector.tensor_reduce(
                                    out=g2_all[:, rt * H:(rt + 1) * H],
                                    in_=g5[:].rearrange("p (t w) -> p w t",
                                                        t=K_TOP),
                                    axis=mybir.AxisListType.X,
                                    op=mybir.AluOpType.add)
                        if debug and rep == 0 and phase_lim >= 6:
                            nc.sync.dma_start(out=dbg_m16[:], in_=m16[:])
                            nc.sync.dma_start(out=dbg_i16[:], in_=i16[:])

                if phase_lim >= 6:
                    # ===== P6: out = relu(0.5*(agg+g2) @ W + b) =====
                    # (0.5 is folded into W host-side)
                    with tc.tile_pool(name=f"p6_{rep}", bufs=1) as p6, \
                         tc.tile_pool(name=f"p6ps_{rep}", bufs=1,
                                      space="PSUM") as p6ps:
                        zf = p6.tile([P, chunks * H], F32)
                        nc.vector.tensor_tensor(out=zf[:], in0=agg_all[:],
                                                in1=g2_all[:],
                                                op=mybir.AluOpType.add)
                        psT = p6ps.tile([P, chunks * HKT * P], F32,
                                        space="PSUM", name="psTz")
                        for j in range(chunks * HKT):
                            nc.tensor.transpose(
                                out=psT[:, j * P:(j + 1) * P],
                                in_=zf[:, j * P:(j + 1) * P],
                                identity=ident[:])
                        zT = p6.tile([P, chunks * HKT * P], F32)
                        nc.scalar.copy(out=zT[:], in_=psT[:])
                        psy = p6ps.tile([P, chunks * C_OUT], F32,
                                        space="PSUM", name="psy")
                        for rt in range(chunks):
                            for k in range(HKT):
                                nc.tensor.matmul(
                                    out=psy[:, rt * C_OUT:(rt + 1) * C_OUT],
                                    lhsT=zT[:, (rt * HKT + k) * P:
                                            (rt * HKT + k + 1) * P],
                                    rhs=gcnw[:, k * C_OUT:(k + 1) * C_OUT],
                                    start=(k == 0), stop=(k == HKT - 1))
                        ot = p6.tile([P, chunks * C_OUT], F32)
                        nc.scalar.copy(out=ot[:], in_=psy[:])
                        o3 = ot[:].rearrange("p (r w) -> p r w", r=chunks)
                        nc.vector.tensor_tensor(
                            out=o3, in0=o3,
                            in1=bias[:, None, :].to_broadcast([P, chunks, C_OUT]),
                            op=mybir.AluOpType.add)
                        nc.vector.tensor_scalar(out=ot[:], in0=ot[:],
                                                scalar1=0.0, scalar2=None,
                                                op0=mybir.AluOpType.max)
                        nc.sync.dma_start(
                            out=out_d[:].rearrange("(r p) w -> p r w", p=P),
                            in_=o3)

            if phase_lim < 6:
                with tc.tile_pool(name="dummyout", bufs=1) as dop:
                    zz = dop.tile([P, C_OUT], F32)
                    nc.vector.memset(zz[:], 0.0)
                    for rt in range(chunks):
                        nc.sync.dma_start(out=out_d[rt * P:(rt + 1) * P, :],
                                          in_=zz[:])

    return nc


def prep_inputs(features, adj_rows, adj_cols, adj_vals, tokens, wp_weight,
                global_token, pre_token_w, combine_w, balance_w, gcn_w, gcn_b,
                n=N):
    """Host-side sharding: row-block features, pad each destination row's
    edge list to T slots, pre-broadcast the small parameters."""
    rows_per_core = n // NCORES
    chunks = rows_per_core // P

    features = np.ascontiguousarray(np.asarray(features, dtype=np.float32))
    r = np.asarray(adj_rows).astype(np.int64)
    c = np.asarray(adj_cols).astype(np.int64)
    v = np.asarray(adj_vals, dtype=np.float32)
    E = r.shape[0]

    pre_token = (np.asarray(wp_weight, np.float32) @
                 np.asarray(tokens, np.float32)).reshape(-1)       # [H]
    cw = np.asarray(combine_w, np.float32).reshape(-1)
    cA = (cw[0] * np.asarray(global_token, np.float32)).reshape(-1)
    cB = pre_token
    cC = (cw[1] * np.asarray(pre_token_w, np.float32)).reshape(-1)
    bal = np.asarray(balance_w, np.float32).reshape(-1)
    bias = np.asarray(gcn_b, np.float32).reshape(-1)

    bcast = lambda x: np.ascontiguousarray(np.tile(x[None, :], (P, 1)))
    cA_b, cB_b, cC_b = bcast(cA), bcast(cB), bcast(cC)
    bal_b, bias_b = bcast(bal), bcast(bias)
    gcnw = np.ascontiguousarray(0.5 * np.asarray(gcn_w, np.float32))
    ident = np.eye(P, dtype=np.float32)

    # dense adjacency (dups coalesce), blocked per core/chunk/k for lhsT:
    # ablk[core][ci, k*128+s, d] = A[core*1024 + ci*128 + d, k*128 + s]
    A = np.zeros((n, n), np.float32)
    np.add.at(A, (r, c), v)
    nkb = n // P
    T = 1

    in_maps = []
    for core in range(NCORES):
        blk = A[core * rows_per_core:(core + 1) * rows_per_core, :]
        t = blk.reshape(chunks, P, nkb, P).transpose(0, 2, 3, 1)
        ablk = np.ascontiguousarray(t.reshape(chunks, nkb * P, P))
        in_maps.append({
            "feat": features[core * rows_per_core:(core + 1) * rows_per_core],
            "ablk": ablk,
            "cA": cA_b, "cB": cB_b, "cC": cC_b, "bal": bal_b, "bias": bias_b,
            "gcnw": gcnw, "ident": ident,
        })
    return in_maps, T


_BUILD_CACHE = {}


def kernel(features, adj_rows, adj_cols, adj_vals, down_k,
           tokens, wp_weight, global_token, pre_token_w, combine_w,
           balance_w, gcn_w, gcn_b):
    k = int(np.asarray(down_k))
    assert k == K_TOP, f"kernel hardcodes top-k={K_TOP}, got {k}"
    in_maps, T = prep_inputs(features, adj_rows, adj_cols, adj_vals, tokens,
                             wp_weight, global_token, pre_token_w, combine_w,
                             balance_w, gcn_w, gcn_b)
    if T not in _BUILD_CACHE:
        nc_new = build(T)
        _split_waits(nc_new)   # hardware-only fixup; breaks CoreSim if applied
        _BUILD_CACHE[T] = nc_new
    nc = _BUILD_CACHE[T]
    res = run_bass_kernel_spmd(nc, in_maps, list(range(NCORES)))
    out = np.concatenate([res.results[i]["out"] for i in range(NCORES)], axis=0)
    return out.astype(np.float32)


# revision 13
# speedup vs baseline: 522.0291x; 4.3456x over previous
"""Trainium2 Bass kernel for nn_DownModel (GNN message passing + kNN graph + GCN).

Math (from the reference):
  f1   = elu(c00*global*relu(pre_token*feat) + c01*pre_token_w*feat)      [N,H]
  agg  = scatter-add over E edges of adj_vals * f1[adj_cols]              [N,H]
  emb  = normalize_rows(balance_w * [f1 | agg])                           [N,2H]
  sim  = emb @ emb.T ; vals,idx = top_k(sim, 16)                          [N,N]
  g2   = sum_j relu(vals_j) * f1[idx_j]                                   [N,H]
  out  = relu(0.5*(agg + g2) @ gcn_w + b)                                 [N,C]
  (uses A@y = (A@f1)@W and re@y = (re@f1)@W to defer the GCN matmul)

This environment is instruction-dispatch-bound (~25-90us per instruction
regardless of size), so the design minimizes instruction count:
  - segment-sum = one 64-index gather + scale + strided tensor_reduce per
    128-row chunk (edges padded per *row* to T slots host-side)
  - top-k gather = one 16-index gather per row-tile
  - sim row-block matmul in f32 (low precision provably flips top-16
    membership and blows the tolerance)
"""

import numpy as np

import concourse.bass as bass
import concourse.mybir as mybir
import concourse.tile as tile
from concourse.bass_utils import run_bass_kernel_spmd

F32 = mybir.dt.float32
I32 = mybir.dt.int32
U32 = mybir.dt.uint32

N = 8192
H = 256
H2 = 2 * H
C_OUT = 40
K_TOP = 16
NCORES = 8
P = 128
ROWS_PER_CORE = N // NCORES          # 1024
CHUNKS = ROWS_PER_CORE // P          # 8
KT = H2 // P                         # 4 k-tiles of the 2H embedding dim
HKT = H // P                         # 2 k-tiles of the H dim
SIMW = 512                           # sim column chunk (one PSUM bank)
NCC = N // SIMW                      # 16 column chunks
GROUP = 2                            # row-tiles sharing one rhs stream pass
CCG = 4                              # column chunks per rhs load


def _split_waits(nc, maxw=1):
    """This container's walrus only accepts one sync-wait command per
    instruction; hoist excess waits onto preceding same-engine NOPs."""
    n_new = 0
    for bb in nc.main_func.blocks:
        new_insts = []
        for ins in bb.instructions:
            si = ins.sync_info
            if si is not None and si.on_wait and len(si.on_wait) > maxw:
                waits = list(si.on_wait)
                excess, keep = waits[:-maxw], waits[-maxw:]
                for i in range(0, len(excess), maxw):
                    nop = mybir.InstNoOp(
                        name=f"waitnop-{ins.name}-{i}",
                        engine=ins.engine,
                        ins=[],
                        outs=[],
                        sync_info=mybir.SyncInfo(
                            on_wait=excess[i:i + maxw], on_update=[]
                        ),
                    )
                    new_insts.append(nop)
                    n_new += 1
                si.on_wait = keep
            new_insts.append(ins)
        bb.instructions[:] = new_insts
    return n_new


def build(T, n=N, debug=False, repeat=1, phase_lim=9):
    """Build the SPMD program (identical on all cores; data differs).

    T: padded edge-slot count per destination row (host-computed max).
    """
    rows_per_core = n // NCORES
    chunks = rows_per_core // P
    ngroups = chunks // GROUP

    nc = bass.Bass(num_devices=NCORES)
    if debug:
        dbg_f1g = nc.dram_tensor("dbg_f1g", [n, H], F32, kind="ExternalOutput")
        dbg_agg = nc.dram_tensor("dbg_agg", [P, chunks * H], F32,
                                 kind="ExternalOutput")
        dbg_m16 = nc.dram_tensor("dbg_m16", [P, chunks * K_TOP], F32,
                                 kind="ExternalOutput")
        dbg_i16 = nc.dram_tensor("dbg_i16", [P, chunks * K_TOP], U32,
                                 kind="ExternalOutput")

    # ---- per-core inputs ----
    feat_d = nc.dram_tensor("feat", [rows_per_core, H], F32, kind="ExternalInput")
    # dense adjacency blocks for this core's rows, transposed per k-block:
    # ablk[ci, k*128+s, d] = A[core*1024 + ci*128 + d, k*128 + s]
    nkb = n // P
    ablk_d = nc.dram_tensor("ablk", [chunks, nkb * P, P], F32,
                            kind="ExternalInput")
    # replicated small params (host pre-broadcast across partitions)
    cA_d = nc.dram_tensor("cA", [P, H], F32, kind="ExternalInput")
    cB_d = nc.dram_tensor("cB", [P, H], F32, kind="ExternalInput")
    cC_d = nc.dram_tensor("cC", [P, H], F32, kind="ExternalInput")
    bal_d = nc.dram_tensor("bal", [P, H2], F32, kind="ExternalInput")
    bias_d = nc.dram_tensor("bias", [P, C_OUT], F32, kind="ExternalInput")
    gcnw_d = nc.dram_tensor("gcnw", [H, C_OUT], F32, kind="ExternalInput")
    ident_d = nc.dram_tensor("ident", [P, P], F32, kind="ExternalInput")

    out_d = nc.dram_tensor("out", [rows_per_core, C_OUT], F32, kind="ExternalOutput")

    with tile.TileContext(nc) as tc:
        with tc.tile_pool(name="consts", bufs=1) as cp, \
             tc.tile_pool(name="persist", bufs=1) as pp, \
             tc.tile_pool(name="dram", bufs=1, space="DRAM") as dp:

            # ---- constants into SBUF ----
            cA = cp.tile([P, H], F32)
            cB = cp.tile([P, H], F32)
            cC = cp.tile([P, H], F32)
            bal = cp.tile([P, H2], F32)
            bias = cp.tile([P, C_OUT], F32)
            gcnw = cp.tile([P, HKT * C_OUT], F32)
            ident = cp.tile([P, P], F32)
            nc.sync.dma_start(out=cA[:], in_=cA_d[:])
            nc.sync.dma_start(out=cB[:], in_=cB_d[:])
            nc.sync.dma_start(out=cC[:], in_=cC_d[:])
            nc.sync.dma_start(out=bal[:], in_=bal_d[:])
            nc.sync.dma_start(out=bias[:], in_=bias_d[:])
            nc.sync.dma_start(
                out=gcnw[:].rearrange("p (k c) -> p k c", k=HKT),
                in_=gcnw_d[:].rearrange("(k p) c -> p k c", p=P))
            nc.sync.dma_start(out=ident[:], in_=ident_d[:])

            # ---- persistent SBUF tensors ----
            g2_all = pp.tile([P, chunks * H], F32)      # re_adj @ f1
            embTloc = pp.tile([P, KT * rows_per_core], F32)  # own embT (lhsT)
            m16 = pp.tile([P, chunks * K_TOP], F32)
            i16 = pp.tile([P, chunks * K_TOP], U32)
            v16 = pp.tile([P, chunks * K_TOP], F32)

            # double-buffered across reps so rep r+1's P1/AG1/P2 overlap
            # rep r's sim phase
            pp2_ctx = tc.tile_pool(name="perrep", bufs=2)
            pp2 = pp2_ctx.__enter__()

            for rep in range(repeat):
                f1_all = pp2.tile([P, chunks * H], F32, tag="f1a",
                                  name=f"f1_all_{rep}")
                agg_all = pp2.tile([P, chunks * H], F32, tag="agga",
                                   name=f"agg_all_{rep}")
                # ---- DRAM scratch (per rep: Shared tensors single-writer) ----
                f1loc = dp.tile([rows_per_core, H], F32, name=f"f1loc_{rep}")
                f1g = dp.tile([n, H], F32, addr_space="Shared",
                              name=f"f1g_{rep}")
                embT_loc_d = dp.tile([H2, rows_per_core], F32,
                                     name=f"embTl_{rep}")
                embT_g = dp.tile([NCORES * H2, rows_per_core], F32,
                                 addr_space="Shared", name=f"embTg_{rep}")

                # ===== P1: f1 for the local row block (batched) =====
                with tc.tile_pool(name=f"p1_{rep}", bufs=1) as p1:
                    W1 = chunks * H
                    ft = p1.tile([P, W1], F32)
                    nc.sync.dma_start(
                        out=ft[:].rearrange("p (r w) -> p r w", r=chunks),
                        in_=feat_d[:].rearrange("(r p) w -> p r w", p=P))
                    mB = lambda c: c[:, None, :].to_broadcast([P, chunks, H])
                    v3 = lambda t: t[:].rearrange("p (r w) -> p r w", r=chunks)
                    m = p1.tile([P, W1], F32)
                    nc.vector.tensor_tensor(out=v3(m), in0=v3(ft), in1=mB(cB),
                                            op=mybir.AluOpType.mult)
                    nc.vector.tensor_scalar(out=m[:], in0=m[:], scalar1=0.0,
                                            scalar2=None, op0=mybir.AluOpType.max)
                    nc.vector.tensor_tensor(out=v3(m), in0=v3(m), in1=mB(cA),
                                            op=mybir.AluOpType.mult)
                    v = p1.tile([P, W1], F32)
                    nc.vector.tensor_tensor(out=v3(v), in0=v3(ft), in1=mB(cC),
                                            op=mybir.AluOpType.mult)
                    nc.vector.tensor_tensor(out=m[:], in0=m[:], in1=v[:],
                                            op=mybir.AluOpType.add)
                    # elu(z) = (relu(z) - 1) + exp(min(z, 0))
                    nc.vector.tensor_scalar(out=v[:], in0=m[:], scalar1=0.0,
                                            scalar2=-1.0, op0=mybir.AluOpType.max,
                                            op1=mybir.AluOpType.add)
                    nc.vector.tensor_scalar(out=m[:], in0=m[:], scalar1=0.0,
                                            scalar2=None, op0=mybir.AluOpType.min)
                    nc.scalar.activation(out=m[:], in_=m[:],
                                         func=mybir.ActivationFunctionType.Exp)
                    nc.vector.tensor_tensor(out=f1_all[:], in0=v[:], in1=m[:],
                                            op=mybir.AluOpType.add)
                    nc.sync.dma_start(
                        out=f1loc[:].rearrange("(r p) w -> p r w", p=P),
                        in_=v3(f1_all))

                if phase_lim >= 2:
                    # ===== P1b: all-gather f1 =====
                    nc.gpsimd.collective_compute(
                        "AllGather", mybir.AluOpType.bypass,
                        replica_groups=[list(range(NCORES))],
                        ins=[f1loc.opt()], outs=[f1g.opt()],
                    )
                    if debug and rep == 0:
                        nc.sync.dma_start(out=dbg_f1g[:], in_=f1g[:])

                if phase_lim >= 3:
                    # ===== P2: segment-sum agg = A @ f1, dense-blocked =====
                    # lhsT = host-built A^T [128 src, 128 dst] blocks, rhs =
                    # gathered f1 k-blocks; 64-deep PSUM chain per chunk.
                    with tc.tile_pool(name=f"p2f_{rep}", bufs=1) as p2f, \
                         tc.tile_pool(name=f"p2a_{rep}", bufs=2) as p2a, \
                         tc.tile_pool(name=f"p2ps_{rep}", bufs=2,
                                      space="PSUM") as p2ps:
                        f1r = p2f.tile([P, nkb * H], F32)
                        nc.sync.dma_start(
                            out=f1r[:].rearrange("p (k w) -> p k w", k=nkb),
                            in_=f1g[:].rearrange("(k p) w -> p k w", p=P))
                        for ci in range(chunks):
                            at = p2a.tile([P, nkb * P], F32, tag="at",
                                          name=f"at_{ci}")
                            nc.sync.dma_start(
                                out=at[:].rearrange("p (k d) -> p k d", k=nkb),
                                in_=ablk_d[ci].rearrange("(k p) d -> p k d",
                                                         p=P))
                            psa = p2ps.tile([P, H], F32, space="PSUM",
                                            tag="psa")
                            for k in range(nkb):
                                nc.tensor.matmul(
                                    out=psa[:],
                                    lhsT=at[:, k * P:(k + 1) * P],
                                    rhs=f1r[:, k * H:(k + 1) * H],
                                    start=(k == 0), stop=(k == nkb - 1))
                            nc.scalar.copy(out=agg_all[:, ci * H:(ci + 1) * H],
                                           in_=psa[:])
                    if debug and rep == 0:
                        nc.sync.dma_start(out=dbg_agg[:], in_=agg_all[:])

                if phase_lim >= 4:
                    # ===== P3: embedding build + transpose + all-gather =====
                    with tc.tile_pool(name=f"p3_{rep}", bufs=1) as p3, \
                         tc.tile_pool(name=f"p3ps_{rep}", bufs=2,
                                      space="PSUM") as p3ps:
                        W3 = chunks * H2
                        zc = p3.tile([P, W3], F32)
                        zc3 = zc[:].rearrange("p (r w) -> p r w", r=chunks)
                        nc.vector.tensor_tensor(
                            out=zc3[:, :, 0:H],
                            in0=f1_all[:].rearrange("p (r w) -> p r w", r=chunks),
                            in1=bal[:, None, 0:H].to_broadcast([P, chunks, H]),
                            op=mybir.AluOpType.mult)
                        nc.vector.tensor_tensor(
                            out=zc3[:, :, H:H2],
                            in0=agg_all[:].rearrange("p (r w) -> p r w", r=chunks),
                            in1=bal[:, None, H:H2].to_broadcast([P, chunks, H]),
                            op=mybir.AluOpType.mult)
                        sq = p3.tile([P, W3], F32)
                        nc.vector.tensor_tensor(out=sq[:], in0=zc[:], in1=zc[:],
                                                op=mybir.AluOpType.mult)
                        n2 = p3.tile([P, chunks], F32)
                        nc.vector.tensor_reduce(
                            out=n2[:],
                            in_=sq[:].rearrange("p (r w) -> p r w", r=chunks),
                            axis=mybir.AxisListType.X,
                            op=mybir.AluOpType.add)
                        nc.scalar.activation(out=n2[:], in_=n2[:],
                                             func=mybir.ActivationFunctionType.Sqrt)
                        inv = p3.tile([P, chunks], F32)
                        nc.vector.reciprocal(out=inv[:], in_=n2[:])
                        nc.vector.tensor_tensor(
                            out=zc3, in0=zc3,
                            in1=inv[:, :, None].to_broadcast([P, chunks, H2]),
                            op=mybir.AluOpType.mult)
                        # transpose all chunks: [128, rt*512 + k*128] blocks
                        for half in range(2):
                            psT = p3ps.tile([P, 4 * KT * P], F32, space="PSUM",
                                            tag="psT")
                            for j in range(4 * KT):
                                rt = half * 4 + j // KT
                                k = j % KT
                                nc.tensor.transpose(
                                    out=psT[:, j * P:(j + 1) * P],
                                    in_=zc[:, rt * H2 + k * P:
                                           rt * H2 + (k + 1) * P],
                                    identity=ident[:])
                            # embTloc layout: [p, k*rows + rt*128 + r]
                            nc.scalar.copy(
                                out=embTloc[:]
                                .rearrange("p (k r) -> p k r", k=KT)
                                [:, :, half * 4 * P:(half + 1) * 4 * P]
                                .rearrange("p k (rt r) -> p rt k r", rt=4),
                                in_=psT[:].rearrange("p (rt k r) -> p rt k r",
                                                     rt=4, k=KT))
                        # one DMA: embT_loc_d[k*128+p, r] = embTloc[p, k*rows+r]
                        nc.sync.dma_start(
                            out=embT_loc_d[:].rearrange("(k p) r -> p k r", p=P),
                            in_=embTloc[:].rearrange("p (k r) -> p k r", k=KT))

                    # ===== P3b: all-gather transposed embedding =====
                    nc.gpsimd.collective_compute(
                        "AllGather", mybir.AluOpType.bypass,
                        replica_groups=[list(range(NCORES))],
                        ins=[embT_loc_d.opt()], outs=[embT_g.opt()],
                    )

                if phase_lim >= 5:
                    # ===== P4: sim row-block + top-k + gather-combine =====
                    with tc.tile_pool(name=f"p4rhs_{rep}", bufs=2) as p4rhs, \
                         tc.tile_pool(name=f"p4sim_{rep}", bufs=GROUP) as p4sim, \
                         tc.tile_pool(name=f"p4g_{rep}", bufs=1) as p4g, \
                         tc.tile_pool(name=f"p4ps_{rep}", bufs=2,
                                      space="PSUM") as p4ps:
                        for g in range(ngroups):
                            rts = [g * GROUP + j for j in range(GROUP)]
                            sims = {rt: p4sim.tile([P, n], F32, tag="simbuf",
                                                   name=f"sim_rt{rt}")
                                    for rt in rts}
                            for cg in range(NCC // CCG):
                                # rhs covers 4 cc = 2048 global cols, all k.
                                # global col block c2 spans core (2cg+c2)'s rows
                                rh = p4rhs.tile([P, KT * CCG * SIMW], F32,
                                                tag="rhs")
                                rh4 = rh[:].rearrange(
                                    "p (k c2 w) -> p k c2 w", k=KT, c2=2)
                                for c2 in range(2):
                                    blk = (2 * cg + c2) * H2
                                    nc.sync.dma_start(
                                        out=rh4[:, :, c2, :],
                                        in_=embT_g[blk:blk + H2, :].rearrange(
                                            "(k p) r -> p k r", p=P))
                                for rt in rts:
                                    pss = p4ps.tile([P, CCG * SIMW], F32,
                                                    space="PSUM", tag="pss")
                                    for q in range(CCG):
                                        for k in range(KT):
                                            nc.tensor.matmul(
                                                out=pss[:, q * SIMW:
                                                        (q + 1) * SIMW],
                                                lhsT=embTloc[
                                                    :, k * rows_per_core
                                                    + rt * P:
                                                    k * rows_per_core
                                                    + (rt + 1) * P],
                                                rhs=rh[:, (k * CCG + q) * SIMW:
                                                       (k * CCG + q + 1) * SIMW],
                                                start=(k == 0),
                                                stop=(k == KT - 1))
                                    nc.scalar.copy(
                                        out=sims[rt][:, cg * CCG * SIMW:
                                                     (cg + 1) * CCG * SIMW],
                                        in_=pss[:])
                            if phase_lim < 6:
                                for rt in rts:
                                    mx = p4g.tile([P, 8], F32, tag="mx")
                                    nc.vector.max(out=mx[:], in_=sims[rt][:])
                                continue
                            for rt in rts:
                                sim = sims[rt]
                                sl16 = slice(rt * K_TOP, (rt + 1) * K_TOP)
                                sl8a = slice(rt * K_TOP, rt * K_TOP + 8)
                                sl8b = slice(rt * K_TOP + 8, (rt + 1) * K_TOP)
                                nc.vector.max(out=m16[:, sl8a], in_=sim[:])
                                nc.vector.max_index(out=i16[:, sl8a],
                                                    in_max=m16[:, sl8a],
                                                    in_values=sim[:])
                                nc.vector.match_replace(out=sim[:],
                                                        in_to_replace=m16[:, sl8a],
                                                        in_values=sim[:],
                                                        imm_value=-1e30)
                                nc.vector.max(out=m16[:, sl8b], in_=sim[:])
                                nc.vector.max_index(out=i16[:, sl8b],
                                                    in_max=m16[:, sl8b],
                                                    in_values=sim[:])
                                # P5: g2 = sum_j relu(v_j) * f1[idx_j]
                                nc.vector.tensor_scalar(
                                    out=v16[:, sl16], in0=m16[:, sl16],
                                    scalar1=0.0, scalar2=None,
                                    op0=mybir.AluOpType.max)
                                g5 = p4g.tile([P, K_TOP * H], F32, tag="g5",
                                              name=f"g5_{rt}")
                                for j in range(K_TOP):
                                    nc.gpsimd.indirect_dma_start(
                                        out=g5[:, j * H:(j + 1) * H],
                                        out_offset=None,
                                        in_=f1g[:, :],
                                        in_offset=bass.IndirectOffsetOnAxis(
                                            ap=i16[:, rt * K_TOP + j:
                                                   rt * K_TOP + j + 1], axis=0),
                                    )
                                nc.vector.tensor_tensor(
                                    out=g5[:].rearrange("p (t w) -> p t w",
                                                        t=K_TOP),
                                    in0=g5[:].rearrange("p (t w) -> p t w",
                                                        t=K_TOP),
                                    in1=v16[:, sl16, None]
                                        .to_broadcast([P, K_TOP, H]),
                                    op=mybir.AluOpType.mult)
                                nc.vector.tensor_reduce(
                                    out=g2_all[:, rt * H:(rt + 1) * H],
                                    in_=g5[:].rearrange("p (t w) -> p w t",
                                                        t=K_TOP),
                                    axis=mybir.AxisListType.X,
                                    op=mybir.AluOpType.add)
                        if debug and rep == 0 and phase_lim >= 6:
                            nc.sync.dma_start(out=dbg_m16[:], in_=m16[:])
                            nc.sync.dma_start(out=dbg_i16[:], in_=i16[:])

                if phase_lim >= 6:
                    # ===== P6: out = relu(0.5*(agg+g2) @ W + b) =====
                    # (0.5 is folded into W host-side)
                    with tc.tile_pool(name=f"p6_{rep}", bufs=1) as p6, \
                         tc.tile_pool(name=f"p6ps_{rep}", bufs=1,
                                      space="PSUM") as p6ps:
                        zf = p6.tile([P, chunks * H], F32)
                        nc.vector.tensor_tensor(out=zf[:], in0=agg_all[:],
                                                in1=g2_all[:],
                                                op=mybir.AluOpType.add)
                        psT = p6ps.tile([P, chunks * HKT * P], F32,
                                        space="PSUM", name="psTz")
                        for j in range(chunks * HKT):
                            nc.tensor.transpose(
                                out=psT[:, j * P:(j + 1) * P],
                                in_=zf[:, j * P:(j + 1) * P],
                                identity=ident[:])
                        zT = p6.tile([P, chunks * HKT * P], F32)
                        nc.scalar.copy(out=zT[:], in_=psT[:])
                        psy = p6ps.tile([P, chunks * C_OUT], F32,
                                        space="PSUM", name="psy")
                        for rt in range(chunks):
                            for k in range(HKT):
                                nc.tensor.matmul(
                                    out=psy[:, rt * C_OUT:(rt + 1) * C_OUT],
                                    lhsT=zT[:, (rt * HKT + k) * P:
                                            (rt * HKT + k + 1) * P],
                                    rhs=gcnw[:, k * C_OUT:(k + 1) * C_OUT],
                                    start=(k == 0), stop=(k == HKT - 1))
                        ot = p6.tile([P, chunks * C_OUT], F32)
                        nc.scalar.copy(out=ot[:], in_=psy[:])
                        o3 = ot[:].rearrange("p (r w) -> p r w", r=chunks)
                        nc.vector.tensor_tensor(
                            out=o3, in0=o3,
                            in1=bias[:, None, :].to_broadcast([P, chunks, C_OUT]),
                            op=mybir.AluOpType.add)
                        nc.vector.tensor_scalar(out=ot[:], in0=ot[:],
                                                scalar1=0.0, scalar2=None,
                                                op0=mybir.AluOpType.max)
                        nc.sync.dma_start(
                            out=out_d[:].rearrange("(r p) w -> p r w", p=P),
                            in_=o3)

            pp2_ctx.__exit__(None, None, None)

            if phase_lim < 6:
                with tc.tile_pool(name="dummyout", bufs=1) as dop:
                    zz = dop.tile([P, C_OUT], F32)
                    nc.vector.memset(zz[:], 0.0)
                    for rt in range(chunks):
                        nc.sync.dma_start(out=out_d[rt * P:(rt + 1) * P, :],
                                          in_=zz[:])

    return nc


def prep_inputs(features, adj_rows, adj_cols, adj_vals, tokens, wp_weight,
                global_token, pre_token_w, combine_w, balance_w, gcn_w, gcn_b,
                n=N):
    """Host-side sharding: row-block features, pad each destination row's
    edge list to T slots, pre-broadcast the small parameters."""
    rows_per_core = n // NCORES
    chunks = rows_per_core // P

    features = np.ascontiguousarray(np.asarray(features, dtype=np.float32))
    r = np.asarray(adj_rows).astype(np.int64)
    c = np.asarray(adj_cols).astype(np.int64)
    v = np.asarray(adj_vals, dtype=np.float32)
    E = r.shape[0]

    pre_token = (np.asarray(wp_weight, np.float32) @
                 np.asarray(tokens, np.float32)).reshape(-1)       # [H]
    cw = np.asarray(combine_w, np.float32).reshape(-1)
    cA = (cw[0] * np.asarray(global_token, np.float32)).reshape(-1)
    cB = pre_token
    cC = (cw[1] * np.asarray(pre_token_w, np.float32)).reshape(-1)
    bal = np.asarray(balance_w, np.float32).reshape(-1)
    bias = np.asarray(gcn_b, np.float32).reshape(-1)

    bcast = lambda x: np.ascontiguousarray(np.tile(x[None, :], (P, 1)))
    cA_b, cB_b, cC_b = bcast(cA), bcast(cB), bcast(cC)
    bal_b, bias_b = bcast(bal), bcast(bias)
    gcnw = np.ascontiguousarray(0.5 * np.asarray(gcn_w, np.float32))
    ident = np.eye(P, dtype=np.float32)

    # dense adjacency (dups coalesce), blocked per core/chunk/k for lhsT:
    # ablk[core][ci, k*128+s, d] = A[core*1024 + ci*128 + d, k*128 + s]
    A = np.zeros((n, n), np.float32)
    np.add.at(A, (r, c), v)
    nkb = n // P
    T = 1

    in_maps = []
    for core in range(NCORES):
        blk = A[core * rows_per_core:(core + 1) * rows_per_core, :]
        t = blk.reshape(chunks, P, nkb, P).transpose(0, 2, 3, 1)
        ablk = np.ascontiguousarray(t.reshape(chunks, nkb * P, P))
        in_maps.append({
            "feat": features[core * rows_per_core:(core + 1) * rows_per_core],
            "ablk": ablk,
            "cA": cA_b, "cB": cB_b, "cC": cC_b, "bal": bal_b, "bias": bias_b,
            "gcnw": gcnw, "ident": ident,
        })
    return in_maps, T


_BUILD_CACHE = {}


def kernel(features, adj_rows, adj_cols, adj_vals, down_k,
           tokens, wp_weight, global_token, pre_token_w, combine_w,
           balance_w, gcn_w, gcn_b):
    k = int(np.asarray(down_k))
    assert k == K_TOP, f"kernel hardcodes top-k={K_TOP}, got {k}"
    in_maps, T = prep_inputs(features, adj_rows, adj_cols, adj_vals, tokens,
                             wp_weight, global_token, pre_token_w, combine_w,
                             balance_w, gcn_w, gcn_b)
    if T not in _BUILD_CACHE:
        nc_new = build(T)
        _split_waits(nc_new)   # hardware-only fixup; breaks CoreSim if applied
        _BUILD_CACHE[T] = nc_new
    nc = _BUILD_CACHE[T]
    res = run_bass_kernel_spmd(nc, in_maps, list(range(NCORES)))
    out = np.concatenate([res.results[i]["out"] for i in range(NCORES)], axis=0)
    return out.astype(np.float32)
